# revision 31
# baseline (speedup 1.0000x reference)
"""PoseMetrics (mpjpe / pa_mpjpe / accel_error) Trainium2 Bass kernel.

Full inputs: pred/target [524288, 3, 14] fp32. Output: [3] fp32.

Strategy (pure data parallel, 8 cores x 65536 samples):
  - Layout: 128 partitions x 512 samples/partition, samples innermost so the
    bulk fp16 elementwise work hits the DVE 2x mode. Inputs are converted
    once to persistent fp16 SBUF tiles (with a global 1/sqrt(8) prescale) and
    never re-streamed.
  - The tensor engine (PE) acts as a free accumulator: identity-weight
    matmuls into PSUM replace the j-sum trees (cross-covariance G, joint sums
    SP/ST) and the 3-way coordinate sums for the per-joint norms.
  - Kabsch/SVD is replaced by a closed form: K = H^T H, largest eigenvalue
    via cubic Newton (Cardano-bound start, 2 iters), remaining eigenvalues by
    quadratic deflation, lambda = s1+s2+sign(det H)*s3, then Markley's FOAM
    formula for R. Slab math fp16, eigen chain fp32.
  - Each core returns [128, 48] partial sums; host reduces in float64.
"""

import numpy as np

import concourse.bass as bass
import concourse.bacc as bacc
import concourse.mybir as mybir
import concourse.tile as tile
from concourse.bass_utils import run_bass_kernel_spmd
from concourse.masks import make_identity

F32 = mybir.dt.float32
F16 = mybir.dt.float16
AX = mybir.AluOpType
AF = mybir.ActivationFunctionType

N_CORES = 8
B_FULL = 524288
B_LOC = B_FULL // N_CORES          # 65536
P = 128                            # partitions
S = B_LOC // P                     # 512 samples per partition
NB = 64                            # samples per chunk (per partition)
NCHUNK = S // NB                   # 8
CJ = 42                            # 3*14
SF = 128                           # FOAM quarter size
SCALE = float(1.0 / np.sqrt(8.0))  # global input prescale (folded out on host)
SQ14I = float(1.0 / np.sqrt(14.0))
NACC = NCHUNK                      # accum slots per metric (1 per chunk)


def _pass1_chunk(nc, loadp, workp, pv, tv, p16, t16, Gp2, Gp1, SPp, STp,
                 n2M, n2A, accM, accA, G16, SPh, STh, I16, ci):
    cs = slice(ci * NB, (ci + 1) * NB)
    x32p = loadp.tile([P, NB, CJ], F32, tag="p32", name="x32p")
    x32t = loadp.tile([P, NB, CJ], F32, tag="t32", name="x32t")
    # fp32 -> fp16 J-major convert with the global prescale folded in.
    # On Pool: ACT is the pass-1 critical engine, Pool is idle here.
    # Chunk 0 is split into quarters so compute starts ~6us earlier.
    nsub = 4 if ci == 0 else 1
    sw = NB // nsub
    for si in range(nsub):
        ls = slice(si * sw, si * sw + sw)
        gs = slice(ci * NB + si * sw, ci * NB + si * sw + sw)
        nc.sync.dma_start(x32p[:, ls, :], pv[:, gs, :])
        nc.sync.dma_start(x32t[:, ls, :], tv[:, gs, :])
        nc.gpsimd.tensor_scalar_mul(
            p16[:, :, :, gs],
            x32p[:, ls, :].rearrange("p s (c j) -> p c j s", c=3, j=14), SCALE)
        nc.gpsimd.tensor_scalar_mul(
            t16[:, :, :, gs],
            x32t[:, ls, :].rearrange("p s (c j) -> p c j s", c=3, j=14), SCALE)

    pcs = p16[:, :, :, cs]
    tcs = t16[:, :, :, cs]

    # ---- mpjpe: d, d^2, PE c-sum, sqrt-accum --------------------------------
    d = workp.tile([P, 3, 14, NB], F16, tag="d", name="d")
    nc.vector.tensor_tensor(d[:], pcs, tcs, op=AX.subtract)
    d2 = workp.tile([P, 3, 14, NB], F16, tag="d2", name="d2")
    nc.scalar.square(d2[:], d[:])
    for sub in range(2):
        ss = slice(sub * 32, sub * 32 + 32)
        ov = n2M[:][:, ss, 0:14].transpose([0, 2, 1])
        for c in range(3):
            nc.tensor.matmul(ov, I16[:], d2[:, c, :, ss],
                             start=(c == 0), stop=(c == 2))
    scrM = workp.tile([P, 64, 14], F16, tag="scrM", name="scrM")
    nc.scalar.activation(scrM[:], n2M[:][:, :, 0:14], AF.Sqrt,
                         accum_out=accM[:, ci:ci + 1])

    # ---- accel: second difference over j, squares, PE c-sum ----------------
    ta = workp.tile([P, 3, 12, NB], F16, tag="ta", name="ta")
    nc.vector.tensor_scalar_mul(ta[:], pcs[:, :, 1:13, :], -2.0)
    nc.vector.tensor_tensor(ta[:], ta[:], pcs[:, :, 0:12, :], op=AX.add)
    nc.vector.tensor_tensor(ta[:], ta[:], pcs[:, :, 2:14, :], op=AX.add)
    a2 = workp.tile([P, 3, 12, NB], F16, tag="a2", name="a2")
    nc.scalar.square(a2[:], ta[:])
    for sub in range(2):
        ss = slice(sub * 32, sub * 32 + 32)
        ov = n2A[:][:, ss, 0:12].transpose([0, 2, 1])
        for c in range(3):
            nc.tensor.matmul(ov, I16[:], a2[:, c, :, ss],
                             start=(c == 0), stop=(c == 2))
    scrA = workp.tile([P, 64, 12], F16, tag="scrA", name="scrA")
    nc.scalar.activation(scrA[:], n2A[:][:, :, 0:12], AF.Sqrt,
                         accum_out=accA[:, ci:ci + 1])

    # ---- G / SP / ST via PE -------------------------------------------------
    # prod[k, i, j, s] = p_i t_k; one TT per k keeps APs within 3 free dims.
    CS = [P, 3, 14, NB]
    prod = workp.tile([P, 3, 3, 14, NB], F16, tag="prod", name="prod")
    for k in range(3):
        nc.vector.tensor_tensor(
            prod[:, k], pcs,
            tcs[:, k].unsqueeze(1).broadcast_to(CS), op=AX.mult)
    # G16[k, i] = sum_j prod[k, i, j]; split k to fit PSUM banks
    for (gp, ksl, nk) in ((Gp2, slice(0, 2), 2), (Gp1, slice(2, 3), 1)):
        for j in range(14):
            nc.tensor.matmul(gp[:], I16[:], prod[:, ksl, :, j, :],
                             start=(j == 0), stop=(j == 13))
    for j in range(14):
        nc.tensor.matmul(SPp[:], I16[:], p16[:, :, j, cs],
                         start=(j == 0), stop=(j == 13))
    for j in range(14):
        nc.tensor.matmul(STp[:], I16[:], t16[:, :, j, cs],
                         start=(j == 0), stop=(j == 13))

    # drains: G + SP/ST on ACT (GPSIMD cannot read PSUM)
    nc.scalar.copy(G16[:, 0:2, :, cs], Gp2[:])
    nc.scalar.copy(G16[:, 2:3, :, cs], Gp1[:])
    nc.scalar.activation(SPh[:, :, cs], SPp[:], AF.Copy, scale=SQ14I)
    nc.scalar.activation(STh[:, :, cs], STp[:], AF.Copy, scale=SQ14I)


def _foam_inv(nc, sp_, chp, G16, SPh, STh, R16, V16, hf):
    """FOAM part A1: H, K, det, invariants.

    H is in s^2 = 1/8 scale (inherited from the input prescale); the FOAM
    formula is scale-invariant so no rescaling is needed anywhere.
    SPh/STh are joint sums scaled by 1/sqrt(14).
    """
    fs = slice(hf * SF, hf * SF + SF)
    S3 = [P, 3, 3, SF]
    # G16 is stored (k, i); present it as (i, k) via a stride view
    Gv = G16[:, :, :, fs].transpose([0, 2, 1, 3])
    SPv = SPh[:, :, fs]
    STv = STh[:, :, fs]

    def slab(name):
        # rotating scratch slab; at most `bufs` of these live at once
        return sp_.tile(S3, F16, tag="ktmp", name=name)

    def ch(name, dt=F32):
        return chp.tile([P, SF], dt, tag="ch32" if dt == F32 else "ch16",
                        name=name)

    def named(tag, dt=F32):
        return chp.tile([P, SF], dt, tag=tag, name=tag, bufs=2)

    # H = G - SP ST^T / 14  (SPh*STh = SP*ST/14 already)
    outer = slab("outer")
    nc.vector.tensor_tensor(
        outer[:], SPv.unsqueeze(2).broadcast_to(S3),
        STv.unsqueeze(1).broadcast_to(S3), op=AX.mult)
    H16 = sp_.tile(S3, F16, tag="H16", bufs=2, name="H16")
    nc.vector.tensor_tensor(H16[:], Gv, outer[:], op=AX.subtract)

    # detH on Pool (fp32 out), from fp16 H
    detH = named("detH")
    c1 = ch("det_c1"); c2 = ch("det_c2"); acc = ch("det_acc")
    nc.gpsimd.tensor_tensor(c1[:], H16[:, 1, 1], H16[:, 2, 2], op=AX.mult)
    nc.gpsimd.tensor_tensor(c2[:], H16[:, 1, 2], H16[:, 2, 1], op=AX.mult)
    nc.gpsimd.tensor_tensor(c1[:], c1[:], c2[:], op=AX.subtract)
    nc.gpsimd.tensor_tensor(acc[:], H16[:, 0, 0], c1[:], op=AX.mult)
    nc.gpsimd.tensor_tensor(c1[:], H16[:, 1, 0], H16[:, 2, 2], op=AX.mult)
    nc.gpsimd.tensor_tensor(c2[:], H16[:, 1, 2], H16[:, 2, 0], op=AX.mult)
    nc.gpsimd.tensor_tensor(c1[:], c1[:], c2[:], op=AX.subtract)
    nc.gpsimd.tensor_tensor(c1[:], H16[:, 0, 1], c1[:], op=AX.mult)
    nc.gpsimd.tensor_tensor(acc[:], acc[:], c1[:], op=AX.subtract)
    nc.gpsimd.tensor_tensor(c1[:], H16[:, 1, 0], H16[:, 2, 1], op=AX.mult)
    nc.gpsimd.tensor_tensor(c2[:], H16[:, 1, 1], H16[:, 2, 0], op=AX.mult)
    nc.gpsimd.tensor_tensor(c1[:], c1[:], c2[:], op=AX.subtract)
    nc.gpsimd.tensor_tensor(c1[:], H16[:, 0, 2], c1[:], op=AX.mult)
    nc.gpsimd.tensor_tensor(detH[:], acc[:], c1[:], op=AX.add)

    # K = H^T H (fp16 slabs, accumulate into K16)
    K16 = sp_.tile(S3, F16, tag="K16", bufs=2, name="K16")
    nc.vector.tensor_tensor(K16[:], H16[:, 0].unsqueeze(2).broadcast_to(S3),
                            H16[:, 0].unsqueeze(1).broadcast_to(S3), op=AX.mult)
    for c in (1, 2):
        tc_ = slab(f"t{c}")
        nc.vector.tensor_tensor(tc_[:], H16[:, c].unsqueeze(2).broadcast_to(S3),
                                H16[:, c].unsqueeze(1).broadcast_to(S3),
                                op=AX.mult)
        nc.vector.tensor_tensor(K16[:], K16[:], tc_[:], op=AX.add)

    # invariants: m2 = tr K (fp32), I3 = detH^2, I2 via Pool
    m2 = named("m2")
    nc.vector.tensor_tensor(m2[:], K16[:, 0, 0], K16[:, 1, 1], op=AX.add)
    nc.vector.tensor_tensor(m2[:], m2[:], K16[:, 2, 2], op=AX.add)
    I3 = named("I3")
    nc.vector.tensor_tensor(I3[:], detH[:], detH[:], op=AX.mult)

    o01 = ch("o01"); o02 = ch("o02"); o12 = ch("o12")
    nc.scalar.square(o01[:], K16[:, 0, 1])
    nc.scalar.square(o02[:], K16[:, 0, 2])
    nc.scalar.square(o12[:], K16[:, 1, 2])
    I2 = named("I2"); mm = ch("mm")
    nc.gpsimd.tensor_tensor(I2[:], K16[:, 0, 0], K16[:, 1, 1], op=AX.mult)
    nc.gpsimd.tensor_tensor(I2[:], I2[:], o01[:], op=AX.subtract)
    nc.gpsimd.tensor_tensor(mm[:], K16[:, 0, 0], K16[:, 2, 2], op=AX.mult)
    nc.gpsimd.tensor_tensor(mm[:], mm[:], o02[:], op=AX.subtract)
    nc.gpsimd.tensor_tensor(I2[:], I2[:], mm[:], op=AX.add)
    nc.gpsimd.tensor_tensor(mm[:], K16[:, 1, 1], K16[:, 2, 2], op=AX.mult)
    nc.gpsimd.tensor_tensor(mm[:], mm[:], o12[:], op=AX.subtract)
    nc.gpsimd.tensor_tensor(I2[:], I2[:], mm[:], op=AX.add)

    return {"H16": H16, "K16": K16, "detH": detH, "m2": m2, "I2": I2,
            "I3": I3, "o01": o01, "o02": o02, "o12": o12}


def _foam_chain(nc, sp_, chp, st, hf):
    """FOAM part A2: eigen chain (Cardano start, Newton, deflation), adjH."""
    fs = slice(hf * SF, hf * SF + SF)
    S3 = [P, 3, 3, SF]
    H16 = st["H16"]; K16 = st["K16"]; detH = st["detH"]
    m2 = st["m2"]; I2 = st["I2"]; I3 = st["I3"]
    o01 = st["o01"]; o02 = st["o02"]; o12 = st["o12"]

    def ch(name, dt=F32):
        return chp.tile([P, SF], dt, tag="ch32" if dt == F32 else "ch16",
                        name=name)

    def named(tag, dt=F32):
        return chp.tile([P, SF], dt, tag=tag, name=tag, bufs=2)

    # Cardano upper bound start: x0 = m2/3 + 2*sqrt((dsum + 2*osum)/6)
    q = named("q")
    nc.vector.tensor_scalar_mul(q[:], m2[:], 1.0 / 3.0)
    osum = ch("osum")
    nc.vector.tensor_tensor(osum[:], o01[:], o02[:], op=AX.add)
    nc.vector.tensor_tensor(osum[:], osum[:], o12[:], op=AX.add)
    dsum = ch("dsum"); kd = ch("kd"); kd2 = ch("kd2")
    nc.vector.tensor_tensor(kd[:], K16[:, 0, 0], q[:], op=AX.subtract)
    nc.vector.tensor_tensor(dsum[:], kd[:], kd[:], op=AX.mult)
    nc.vector.tensor_tensor(kd[:], K16[:, 1, 1], q[:], op=AX.subtract)
    nc.vector.tensor_tensor(kd2[:], kd[:], kd[:], op=AX.mult)
    nc.vector.tensor_tensor(dsum[:], dsum[:], kd2[:], op=AX.add)
    nc.vector.tensor_tensor(kd[:], K16[:, 2, 2], q[:], op=AX.subtract)
    nc.vector.tensor_tensor(kd2[:], kd[:], kd[:], op=AX.mult)
    nc.vector.tensor_tensor(dsum[:], dsum[:], kd2[:], op=AX.add)
    p2 = ch("p2")
    nc.vector.scalar_tensor_tensor(p2[:], osum[:], 2.0, dsum[:],
                                   op0=AX.mult, op1=AX.add)
    pC = ch("pC")
    nc.scalar.activation(pC[:], p2[:], AF.Sqrt, scale=1.0 / 6.0)
    X = named("X")
    nc.vector.scalar_tensor_tensor(X[:], pC[:], 2.0, q[:],
                                   op0=AX.mult, op1=AX.add)

    # Newton on f(x) = ((x - m2) x + I2) x - I3, 2 iters from above
    m2_2 = named("m2_2")
    nc.vector.tensor_scalar_mul(m2_2[:], m2[:], 2.0)
    na = ch("na"); nb = ch("nb")
    for _ in range(2):
        nc.vector.tensor_tensor(na[:], X[:], m2[:], op=AX.subtract)
        nc.vector.tensor_tensor(na[:], na[:], X[:], op=AX.mult)
        nc.vector.tensor_tensor(na[:], na[:], I2[:], op=AX.add)
        nc.vector.tensor_tensor(na[:], na[:], X[:], op=AX.mult)
        nc.vector.tensor_tensor(na[:], na[:], I3[:], op=AX.subtract)   # f
        nc.vector.tensor_scalar_mul(nb[:], X[:], 3.0)
        nc.vector.tensor_tensor(nb[:], nb[:], m2_2[:], op=AX.subtract)
        nc.vector.tensor_tensor(nb[:], nb[:], X[:], op=AX.mult)
        nc.vector.tensor_tensor(nb[:], nb[:], I2[:], op=AX.add)        # f'
        nc.vector.reciprocal(nb[:], nb[:])
        nc.vector.tensor_tensor(na[:], na[:], nb[:], op=AX.mult)
        nc.vector.tensor_tensor(X[:], X[:], na[:], op=AX.subtract)

    # deflate: mu2/mu3 from x^2 - (m2-mu1)x + I3/mu1
    mus = chp.tile([P, 3, SF], F32, tag="mus", name="mus", bufs=2)
    mu1 = mus[:, 0]; mu2 = mus[:, 1]; mu3 = mus[:, 2]
    nc.vector.tensor_scalar_max(mu1, X[:], 1e-25)
    b = ch("b"); cc = ch("cc"); rmu = ch("rmu")
    nc.vector.tensor_tensor(b[:], m2[:], mu1, op=AX.subtract)
    nc.vector.reciprocal(rmu[:], mu1)
    nc.vector.tensor_tensor(cc[:], I3[:], rmu[:], op=AX.mult)
    b2 = ch("b2")
    nc.vector.tensor_tensor(b2[:], b[:], b[:], op=AX.mult)
    disc2 = ch("disc2")
    nc.vector.scalar_tensor_tensor(disc2[:], cc[:], -4.0, b2[:],
                                   op0=AX.mult, op1=AX.add)
    nc.vector.tensor_scalar_max(disc2[:], disc2[:], 0.0)
    disc = ch("disc")
    nc.scalar.sqrt(disc[:], disc2[:])
    bh = ch("bh")
    nc.vector.tensor_scalar_mul(bh[:], b[:], 0.5)
    nc.vector.scalar_tensor_tensor(mu2, disc[:], 0.5, bh[:],
                                   op0=AX.mult, op1=AX.add)
    nc.vector.tensor_scalar_max(mu2, mu2, 0.0)
    nc.vector.tensor_tensor(mu3, b[:], mu2, op=AX.subtract)
    nc.vector.tensor_scalar_max(mu3, mu3, 0.0)

    rt = chp.tile([P, 3, SF], F32, tag="rt", name="rt", bufs=2)
    nc.scalar.sqrt(rt[:], mus[:])
    sgn = ch("sgn")
    nc.scalar.sign(sgn[:], detH[:])
    lam = named("lam")
    nc.vector.tensor_tensor(lam[:], rt[:, 0], rt[:, 1], op=AX.add)
    s3s = ch("s3s")
    nc.vector.tensor_tensor(s3s[:], sgn[:], rt[:, 2], op=AX.mult)
    nc.vector.tensor_tensor(lam[:], lam[:], s3s[:], op=AX.add)

    # alpha2 = lam^2 + m2 ; zeta2 = (lam^2 - m2) lam - 2 detH (floored)
    lam2 = ch("lam2"); alpha2 = named("alpha2")
    nc.vector.tensor_tensor(lam2[:], lam[:], lam[:], op=AX.mult)
    nc.vector.tensor_tensor(alpha2[:], lam2[:], m2[:], op=AX.add)
    zt = ch("zt")
    nc.vector.tensor_tensor(zt[:], lam2[:], m2[:], op=AX.subtract)
    nc.vector.tensor_tensor(zt[:], zt[:], lam[:], op=AX.mult)
    zeta2 = ch("zeta2")
    nc.vector.scalar_tensor_tensor(zeta2[:], detH[:], -2.0, zt[:],
                                   op0=AX.mult, op1=AX.add)
    m2s = ch("m2s")
    nc.scalar.sqrt(m2s[:], m2[:])
    zfl = ch("zfl")
    nc.vector.scalar_tensor_tensor(zfl[:], m2s[:], 1e-4, m2[:],
                                   op0=AX.mult, op1=AX.mult)
    nc.vector.tensor_tensor(zeta2[:], zeta2[:], zfl[:], op=AX.max)
    rz = ch("rz")
    nc.vector.reciprocal(rz[:], zeta2[:])

    # fp16 stage for the slab assembly
    a16 = named("a16", F16)
    nc.vector.tensor_copy(a16[:], alpha2[:])
    l16 = named("l16", F16)
    nc.vector.tensor_scalar_mul(l16[:], lam[:], 2.0)
    rz16 = named("rz16", F16)
    nc.vector.tensor_copy(rz16[:], rz[:])

    # adjugate of H: fp16 channel ops on DVE (cheap in 2x mode)
    adjH = sp_.tile(S3, F16, tag="adjH", bufs=2, name="adjH")
    idx = [
        (0, 0, (1, 1), (2, 2), (1, 2), (2, 1)),
        (0, 1, (0, 2), (2, 1), (0, 1), (2, 2)),
        (0, 2, (0, 1), (1, 2), (0, 2), (1, 1)),
        (1, 0, (1, 2), (2, 0), (1, 0), (2, 2)),
        (1, 1, (0, 0), (2, 2), (0, 2), (2, 0)),
        (1, 2, (0, 2), (1, 0), (0, 0), (1, 2)),
        (2, 0, (1, 0), (2, 1), (1, 1), (2, 0)),
        (2, 1, (0, 1), (2, 0), (0, 0), (2, 1)),
        (2, 2, (0, 0), (1, 1), (0, 1), (1, 0)),
    ]
    aw1 = ch("aw1", F16); aw2 = ch("aw2", F16)
    for (i, j, (a1, a2), (b1, b2), (c1_, c2_), (d1, d2)) in idx:
        nc.vector.tensor_tensor(aw1[:], H16[:, a1, a2], H16[:, b1, b2], op=AX.mult)
        nc.vector.tensor_tensor(aw2[:], H16[:, c1_, c2_], H16[:, d1, d2], op=AX.mult)
        nc.vector.tensor_tensor(adjH[:, i, j], aw1[:], aw2[:], op=AX.subtract)

    return {"H16": H16, "K16": K16, "adjH": adjH,
            "a16": a16, "l16": l16, "rz16": rz16}


def _foam_b(nc, sp_, chp, st, G16, SPh, STh, R16, V16, t16, hf):
    return _foam_half_b(nc, sp_, chp, st, G16, SPh, STh, R16, V16, t16, hf)


def _foam_half_b(nc, sp_, chp, st, G16, SPh, STh, R16, V16, t16, hf):
    """FOAM part B: slab assembly, R, V, and the V fold into t16."""
    fs = slice(hf * SF, hf * SF + SF)
    S3 = [P, 3, 3, SF]
    SPv = SPh[:, :, fs]
    STv = STh[:, :, fs]
    H16 = st["H16"]; K16 = st["K16"]; adjH = st["adjH"]
    a16 = st["a16"]; l16 = st["l16"]; rz16 = st["rz16"]

    def slab(name):
        return sp_.tile(S3, F16, tag="ktmp", name=name)

    # num = (alpha2 I - 2K) H^T + 2 lam adjH ;  R = num / zeta2, clamped
    W = sp_.tile(S3, F16, tag="Mt", bufs=2, name="W")
    nc.vector.tensor_scalar_mul(W[:], K16[:], -2.0)
    # diagonal view: stride 4*SF within the contiguous [3,3,SF] block
    nc.vector.tensor_tensor(
        W[:].rearrange("p a b s -> p (a b) s")[:, 0:9:4, :],
        W[:].rearrange("p a b s -> p (a b) s")[:, 0:9:4, :],
        a16[:].unsqueeze(1).broadcast_to([P, 3, SF]), op=AX.add)
    Ht = H16[:].transpose([0, 2, 1, 3])
    num = slab("num")
    nc.vector.tensor_tensor(num[:], W[:, :, 0].unsqueeze(2).broadcast_to(S3),
                            H16[:, :, 0].unsqueeze(1).broadcast_to(S3), op=AX.mult)
    for c in (1, 2):
        uc = slab(f"u{c}")
        nc.vector.tensor_tensor(uc[:], W[:, :, c].unsqueeze(2).broadcast_to(S3),
                                H16[:, :, c].unsqueeze(1).broadcast_to(S3),
                                op=AX.mult)
        nc.vector.tensor_tensor(num[:], num[:], uc[:], op=AX.add)
    vB = slab("vB")
    nc.vector.tensor_tensor(
        vB[:], l16[:].unsqueeze(1).unsqueeze(2).broadcast_to(S3), adjH[:],
        op=AX.mult)
    nc.vector.tensor_tensor(num[:], num[:], vB[:], op=AX.add)
    R16v = R16[:, :, :, fs]
    nc.vector.tensor_tensor(
        R16v, num[:], rz16[:].unsqueeze(1).unsqueeze(2).broadcast_to(S3),
        op=AX.mult)
    nc.vector.tensor_scalar(R16v, R16v, 4.0, -4.0, op0=AX.min, op1=AX.max)

    # V = (STh - R SPh) / sqrt(14)  (== t_mean - R p_mean)
    pv_ = slab("pv_")
    nc.vector.tensor_tensor(pv_[:], R16v, SPv.unsqueeze(1).broadcast_to(S3),
                            op=AX.mult)
    RS = chp.tile([P, 3, SF], F16, tag="RS", name="RS", bufs=2)
    nc.vector.tensor_tensor(RS[:], pv_[:, :, 0], pv_[:, :, 1], op=AX.add)
    nc.vector.tensor_tensor(RS[:], RS[:], pv_[:, :, 2], op=AX.add)
    Vt = chp.tile([P, 3, SF], F16, tag="Vt", name="Vt", bufs=2)
    nc.vector.tensor_tensor(Vt[:], STv, RS[:], op=AX.subtract)
    nc.vector.tensor_scalar_mul(V16[:, :, fs], Vt[:], SQ14I)

    # fold V into t16 in place: pass3's residual becomes qv - t16
    TSH = [P, 3, 14, SF]
    nc.vector.tensor_tensor(
        t16[:, :, :, fs], t16[:, :, :, fs],
        V16[:, :, fs].unsqueeze(2).broadcast_to(TSH), op=AX.subtract)


def _pass3_chunk(nc, workp, p16, t16, R16, V16, n2P, dvps, accP, I16, nI16,
                 ci, use_pe):
    cs = slice(ci * NB, (ci + 1) * NB)
    CS = [P, 3, 14, NB]
    # prq[k][i, j, s] = R_ik p_kj
    prqs = []
    for k in range(3):
        prq = workp.tile(CS, F16, tag=f"prq{k}", name=f"prq{k}")
        nc.vector.tensor_tensor(
            prq[:], R16[:, :, k, cs].unsqueeze(2).broadcast_to(CS),
            p16[:, k, :, cs].unsqueeze(1).broadcast_to(CS), op=AX.mult)
        prqs.append(prq)
    dv2 = workp.tile(CS, F16, tag="dv2", name="dv2")
    tcs = t16[:, :, :, cs]
    if use_pe:
        # PE sums over k and subtracts t16 (V already folded into t16)
        subs = [(i * 12, min(12, NB - i * 12)) for i in range((NB + 11) // 12)]
        for si, (s0, sw) in enumerate(subs):
            ss = slice(s0, s0 + sw)
            dvp = dvps[si % len(dvps)]
            for k in range(3):
                nc.tensor.matmul(dvp[:, :, :, 0:sw], I16[:],
                                 prqs[k][:, :, :, ss],
                                 start=(k == 0), stop=False)
            nc.tensor.matmul(dvp[:, :, :, 0:sw], nI16[:], tcs[:, :, :, ss],
                             start=False, stop=True)
            nc.scalar.square(dv2[:, :, :, ss], dvp[:, :, :, 0:sw])
    else:
        # DVE sums (tail chunks: PE is the critical engine there)
        nc.vector.tensor_tensor(prqs[0][:], prqs[0][:], prqs[1][:], op=AX.add)
        nc.vector.tensor_tensor(prqs[0][:], prqs[0][:], prqs[2][:], op=AX.add)
        nc.vector.tensor_tensor(prqs[0][:], prqs[0][:], tcs, op=AX.subtract)
        nc.scalar.square(dv2[:], prqs[0][:])
    for sub in range(2):
        ss = slice(sub * 32, sub * 32 + 32)
        ov = n2P[:][:, ss, 0:14].transpose([0, 2, 1])
        for c in range(3):
            nc.tensor.matmul(ov, I16[:], dv2[:, c, :, ss],
                             start=(c == 0), stop=(c == 2))
    scrP = workp.tile([P, 64, 14], F16, tag="scrP", name="scrP")
    nc.scalar.activation(scrP[:], n2P[:][:, :, 0:14], AF.Sqrt,
                         accum_out=accP[:, ci:ci + 1])


def build_bass():
    nc = bacc.Bacc("TRN2")
    pred = nc.dram_tensor("pred", [B_LOC, CJ], F32, kind="ExternalInput")
    targ = nc.dram_tensor("target", [B_LOC, CJ], F32, kind="ExternalInput")
    out = nc.dram_tensor("out", [P, 3 * NACC], F32, kind="ExternalOutput")

    pv = pred[:].rearrange("(p n) d -> p n d", p=P)   # [128, 512, 42]
    tv = targ[:].rearrange("(p n) d -> p n d", p=P)

    with tile.TileContext(nc) as tc:
        with tc.tile_pool(name="persist", bufs=1) as pp:
            p16 = pp.tile([P, 3, 14, S], F16, tag="p16")
            t16 = pp.tile([P, 3, 14, S], F16, tag="t16")
            G16 = pp.tile([P, 3, 3, S], F16, tag="G16")
            SPh = pp.tile([P, 3, S], F16, tag="SPh")
            STh = pp.tile([P, 3, S], F16, tag="STh")
            R16 = pp.tile([P, 3, 3, S], F16, tag="R16")
            V16 = pp.tile([P, 3, S], F16, tag="V16")
            accM = pp.tile([P, NACC], F32, tag="accM")
            accA = pp.tile([P, NACC], F32, tag="accA")
            accP = pp.tile([P, NACC], F32, tag="accP")
            I16 = pp.tile([P, P], F16, tag="I16")
            make_identity(nc, I16[:])
            nI16 = pp.tile([P, P], F16, tag="nI16")
            nc.vector.tensor_scalar_mul(nI16[:], I16[:], -1.0)

            # ---------------- pass 1 ----------------------------------------
            with tc.tile_pool(name="load1", bufs=2) as loadp, \
                 tc.tile_pool(name="work1", bufs=1) as workp, \
                 tc.tile_pool(name="ps1", bufs=1, space="PSUM") as psp:
                Gp2 = psp.tile([P, 2, 3, NB], F32, tag="Gp2")
                Gp1 = psp.tile([P, 1, 3, NB], F32, tag="Gp1")
                SPp = psp.tile([P, 3, NB], F32, tag="SPp")
                STp = psp.tile([P, 3, NB], F32, tag="STp")
                n2M = psp.tile([P, 64, 16], F32, tag="n2M", name="n2M")
                n2A = psp.tile([P, 64, 16], F32, tag="n2A", name="n2A")
                for ci in range(NCHUNK):
                    _pass1_chunk(nc, loadp, workp, pv, tv, p16, t16,
                                 Gp2, Gp1, SPp, STp, n2M, n2A,
                                 accM, accA, G16, SPh, STh, I16, ci)

            # ---------------- FOAM + pass 3, interleaved --------------------
            with tc.tile_pool(name="slab_a", bufs=2) as sp_a, \
                 tc.tile_pool(name="ch_a", bufs=14) as chp_a, \
                 tc.tile_pool(name="work3", bufs=1) as workp3, \
                 tc.tile_pool(name="ps3", bufs=1, space="PSUM") as psp3:
                n2P = psp3.tile([P, 64, 16], F32, tag="n2P", name="n2P")
                dvps = [psp3.tile([P, 3, 14, 12], F32, tag=f"dvp{s}",
                                  name=f"dvp{s}") for s in range(2)]
                # quarter-pipelined FOAM: pass3 chunks fill the gaps
                def P3(ci, use_pe=True):
                    _pass3_chunk(nc, workp3, p16, t16, R16, V16, n2P, dvps,
                                 accP, I16, nI16, ci, use_pe)
                for qi in range(4):
                    stq = _foam_inv(nc, sp_a, chp_a, G16, SPh, STh,
                                    R16, V16, qi)
                    if qi >= 1:
                        P3(2 * (qi - 1))
                    stq = dict(stq, **_foam_chain(nc, sp_a, chp_a, stq, qi))
                    if qi >= 1:
                        P3(2 * (qi - 1) + 1)
                    _foam_b(nc, sp_a, chp_a, stq, G16, SPh, STh, R16, V16,
                            t16, qi)
                P3(6, True)
                P3(7, False)

            stage = pp.tile([P, 3 * NACC], F32, tag="stage", name="stage")
            nc.gpsimd.tensor_copy(stage[:, 0:NACC], accM[:])
            nc.gpsimd.tensor_copy(stage[:, NACC:2 * NACC], accP[:])
            nc.gpsimd.tensor_copy(stage[:, 2 * NACC:3 * NACC], accA[:])
            nc.sync.dma_start(out[:], stage[:])

    nc.finalize()
    return nc


_NC = None


def kernel(pred: np.ndarray, target: np.ndarray) -> np.ndarray:
    global _NC
    if _NC is None:
        _NC = build_bass()

    pred = np.ascontiguousarray(pred, dtype=np.float32).reshape(B_FULL, CJ)
    target = np.ascontiguousarray(target, dtype=np.float32).reshape(B_FULL, CJ)

    in_maps = []
    for c in range(N_CORES):
        sl = slice(c * B_LOC, (c + 1) * B_LOC)
        in_maps.append({"pred": pred[sl], "target": target[sl]})

    res = run_bass_kernel_spmd(_NC, in_maps, core_ids=list(range(N_CORES)))
    mp = pa = ac = 0.0
    for r in res.results:
        o = r["out"].astype(np.float64)
        mp += o[:, 0:NACC].sum()
        pa += o[:, NACC:2 * NACC].sum()
        ac += o[:, 2 * NACC:3 * NACC].sum()
    inv = 1.0 / SCALE
    return np.array([mp / (B_FULL * 14) * inv,
                     pa / (B_FULL * 14) * inv,
                     ac / (B_FULL * 12) * inv], dtype=np.float32)


# revision 33
# speedup vs baseline: 1.0243x; 1.0243x over previous
"""PoseMetrics (mpjpe / pa_mpjpe / accel_error) Trainium2 Bass kernel.

Full inputs: pred/target [524288, 3, 14] fp32. Output: [3] fp32.

Strategy (pure data parallel, 8 cores x 65536 samples):
  - Layout: 128 partitions x 512 samples/partition, samples innermost so the
    bulk fp16 elementwise work hits the DVE 2x mode. Inputs are converted
    once to persistent fp16 SBUF tiles (with a global 1/sqrt(8) prescale) and
    never re-streamed.
  - The tensor engine (PE) acts as a free accumulator: identity-weight
    matmuls into PSUM replace the j-sum trees (cross-covariance G, joint sums
    SP/ST) and the 3-way coordinate sums for the per-joint norms.
  - Kabsch/SVD is replaced by a closed form: K = H^T H, largest eigenvalue
    via cubic Newton (Cardano-bound start, 2 iters), remaining eigenvalues by
    quadratic deflation, lambda = s1+s2+sign(det H)*s3, then Markley's FOAM
    formula for R. Slab math fp16, eigen chain fp32.
  - Each core returns [128, 48] partial sums; host reduces in float64.
"""

import numpy as np

import concourse.bass as bass
import concourse.bacc as bacc
import concourse.mybir as mybir
import concourse.tile as tile
from concourse.bass_utils import run_bass_kernel_spmd
from concourse.masks import make_identity

F32 = mybir.dt.float32
F16 = mybir.dt.float16
AX = mybir.AluOpType
AF = mybir.ActivationFunctionType

N_CORES = 8
B_FULL = 524288
B_LOC = B_FULL // N_CORES          # 65536
P = 128                            # partitions
S = B_LOC // P                     # 512 samples per partition
NB = 64                            # samples per chunk (per partition)
NCHUNK = S // NB                   # 8
CJ = 42                            # 3*14
SF = 256                           # FOAM half size
SCALE = float(1.0 / np.sqrt(8.0))  # global input prescale (folded out on host)
SQ14I = float(1.0 / np.sqrt(14.0))
NACC = NCHUNK                      # accum slots per metric (1 per chunk)


def _pass1_chunk(nc, loadp, workp, pv, tv, p16, t16, Gp2, Gp1, SPp, STp,
                 n2M, n2A, accM, accA, G16, SPh, STh, I16, ci):
    cs = slice(ci * NB, (ci + 1) * NB)
    x32p = loadp.tile([P, NB, CJ], F32, tag="p32", name="x32p")
    x32t = loadp.tile([P, NB, CJ], F32, tag="t32", name="x32t")
    # fp32 -> fp16 J-major convert with the global prescale folded in.
    # On Pool: ACT is the pass-1 critical engine, Pool is idle here.
    # Chunk 0 is split into quarters so compute starts ~6us earlier.
    nsub = 4 if ci == 0 else 1
    sw = NB // nsub
    for si in range(nsub):
        ls = slice(si * sw, si * sw + sw)
        gs = slice(ci * NB + si * sw, ci * NB + si * sw + sw)
        nc.sync.dma_start(x32p[:, ls, :], pv[:, gs, :])
        nc.sync.dma_start(x32t[:, ls, :], tv[:, gs, :])
        nc.gpsimd.tensor_scalar_mul(
            p16[:, :, :, gs],
            x32p[:, ls, :].rearrange("p s (c j) -> p c j s", c=3, j=14), SCALE)
        nc.gpsimd.tensor_scalar_mul(
            t16[:, :, :, gs],
            x32t[:, ls, :].rearrange("p s (c j) -> p c j s", c=3, j=14), SCALE)

    pcs = p16[:, :, :, cs]
    tcs = t16[:, :, :, cs]

    # ---- mpjpe: d, d^2, PE c-sum, sqrt-accum --------------------------------
    d = workp.tile([P, 3, 14, NB], F16, tag="d", name="d")
    nc.vector.tensor_tensor(d[:], pcs, tcs, op=AX.subtract)
    d2 = workp.tile([P, 3, 14, NB], F16, tag="d2", name="d2")
    nc.scalar.square(d2[:], d[:])
    for sub in range(2):
        ss = slice(sub * 32, sub * 32 + 32)
        ov = n2M[:][:, ss, 0:14].transpose([0, 2, 1])
        for c in range(3):
            nc.tensor.matmul(ov, I16[:], d2[:, c, :, ss],
                             start=(c == 0), stop=(c == 2))
    scrM = workp.tile([P, 64, 14], F16, tag="scrM", name="scrM")
    nc.scalar.activation(scrM[:], n2M[:][:, :, 0:14], AF.Sqrt,
                         accum_out=accM[:, ci:ci + 1])

    # ---- accel: second difference over j, squares, PE c-sum ----------------
    ta = workp.tile([P, 3, 12, NB], F16, tag="ta", name="ta")
    nc.vector.tensor_scalar_mul(ta[:], pcs[:, :, 1:13, :], -2.0)
    nc.vector.tensor_tensor(ta[:], ta[:], pcs[:, :, 0:12, :], op=AX.add)
    nc.vector.tensor_tensor(ta[:], ta[:], pcs[:, :, 2:14, :], op=AX.add)
    a2 = workp.tile([P, 3, 12, NB], F16, tag="a2", name="a2")
    nc.scalar.square(a2[:], ta[:])
    for sub in range(2):
        ss = slice(sub * 32, sub * 32 + 32)
        ov = n2A[:][:, ss, 0:12].transpose([0, 2, 1])
        for c in range(3):
            nc.tensor.matmul(ov, I16[:], a2[:, c, :, ss],
                             start=(c == 0), stop=(c == 2))
    scrA = workp.tile([P, 64, 12], F16, tag="scrA", name="scrA")
    nc.scalar.activation(scrA[:], n2A[:][:, :, 0:12], AF.Sqrt,
                         accum_out=accA[:, ci:ci + 1])

    # ---- G / SP / ST via PE -------------------------------------------------
    # prod[k, i, j, s] = p_i t_k; one TT per k keeps APs within 3 free dims.
    CS = [P, 3, 14, NB]
    prod = workp.tile([P, 3, 3, 14, NB], F16, tag="prod", name="prod")
    for k in range(3):
        nc.vector.tensor_tensor(
            prod[:, k], pcs,
            tcs[:, k].unsqueeze(1).broadcast_to(CS), op=AX.mult)
    # G16[k, i] = sum_j prod[k, i, j]; split k to fit PSUM banks
    for (gp, ksl, nk) in ((Gp2, slice(0, 2), 2), (Gp1, slice(2, 3), 1)):
        for j in range(14):
            nc.tensor.matmul(gp[:], I16[:], prod[:, ksl, :, j, :],
                             start=(j == 0), stop=(j == 13))
    for j in range(14):
        nc.tensor.matmul(SPp[:], I16[:], p16[:, :, j, cs],
                         start=(j == 0), stop=(j == 13))
    for j in range(14):
        nc.tensor.matmul(STp[:], I16[:], t16[:, :, j, cs],
                         start=(j == 0), stop=(j == 13))

    # drains: G + SP/ST on ACT (GPSIMD cannot read PSUM)
    nc.scalar.copy(G16[:, 0:2, :, cs], Gp2[:])
    nc.scalar.copy(G16[:, 2:3, :, cs], Gp1[:])
    nc.scalar.activation(SPh[:, :, cs], SPp[:], AF.Copy, scale=SQ14I)
    nc.scalar.activation(STh[:, :, cs], STp[:], AF.Copy, scale=SQ14I)


def _foam_inv(nc, sp_, chp, G16, SPh, STh, R16, V16, hf):
    """FOAM part A1: H, K, det, invariants.

    H is in s^2 = 1/8 scale (inherited from the input prescale); the FOAM
    formula is scale-invariant so no rescaling is needed anywhere.
    SPh/STh are joint sums scaled by 1/sqrt(14).
    """
    fs = slice(hf * SF, hf * SF + SF)
    S3 = [P, 3, 3, SF]
    # G16 is stored (k, i); present it as (i, k) via a stride view
    Gv = G16[:, :, :, fs].transpose([0, 2, 1, 3])
    SPv = SPh[:, :, fs]
    STv = STh[:, :, fs]

    def slab(name):
        # rotating scratch slab; at most `bufs` of these live at once
        return sp_.tile(S3, F16, tag="ktmp", name=name)

    def ch(name, dt=F32):
        return chp.tile([P, SF], dt, tag="ch32" if dt == F32 else "ch16",
                        name=name)

    def named(tag, dt=F32):
        return chp.tile([P, SF], dt, tag=tag, name=tag, bufs=1)

    # H = G - SP ST^T / 14  (SPh*STh = SP*ST/14 already)
    outer = slab("outer")
    nc.vector.tensor_tensor(
        outer[:], SPv.unsqueeze(2).broadcast_to(S3),
        STv.unsqueeze(1).broadcast_to(S3), op=AX.mult)
    H16 = sp_.tile(S3, F16, tag="H16", bufs=1, name="H16")
    nc.vector.tensor_tensor(H16[:], Gv, outer[:], op=AX.subtract)

    # detH on Pool (fp32 out), from fp16 H
    detH = named("detH")
    c1 = ch("det_c1"); c2 = ch("det_c2"); acc = ch("det_acc")
    nc.gpsimd.tensor_tensor(c1[:], H16[:, 1, 1], H16[:, 2, 2], op=AX.mult)
    nc.gpsimd.tensor_tensor(c2[:], H16[:, 1, 2], H16[:, 2, 1], op=AX.mult)
    nc.gpsimd.tensor_tensor(c1[:], c1[:], c2[:], op=AX.subtract)
    nc.gpsimd.tensor_tensor(acc[:], H16[:, 0, 0], c1[:], op=AX.mult)
    nc.gpsimd.tensor_tensor(c1[:], H16[:, 1, 0], H16[:, 2, 2], op=AX.mult)
    nc.gpsimd.tensor_tensor(c2[:], H16[:, 1, 2], H16[:, 2, 0], op=AX.mult)
    nc.gpsimd.tensor_tensor(c1[:], c1[:], c2[:], op=AX.subtract)
    nc.gpsimd.tensor_tensor(c1[:], H16[:, 0, 1], c1[:], op=AX.mult)
    nc.gpsimd.tensor_tensor(acc[:], acc[:], c1[:], op=AX.subtract)
    nc.gpsimd.tensor_tensor(c1[:], H16[:, 1, 0], H16[:, 2, 1], op=AX.mult)
    nc.gpsimd.tensor_tensor(c2[:], H16[:, 1, 1], H16[:, 2, 0], op=AX.mult)
    nc.gpsimd.tensor_tensor(c1[:], c1[:], c2[:], op=AX.subtract)
    nc.gpsimd.tensor_tensor(c1[:], H16[:, 0, 2], c1[:], op=AX.mult)
    nc.gpsimd.tensor_tensor(detH[:], acc[:], c1[:], op=AX.add)

    # K = H^T H (fp16 slabs, accumulate into K16)
    K16 = sp_.tile(S3, F16, tag="K16", bufs=1, name="K16")
    nc.vector.tensor_tensor(K16[:], H16[:, 0].unsqueeze(2).broadcast_to(S3),
                            H16[:, 0].unsqueeze(1).broadcast_to(S3), op=AX.mult)
    for c in (1, 2):
        tc_ = slab(f"t{c}")
        nc.vector.tensor_tensor(tc_[:], H16[:, c].unsqueeze(2).broadcast_to(S3),
                                H16[:, c].unsqueeze(1).broadcast_to(S3),
                                op=AX.mult)
        nc.vector.tensor_tensor(K16[:], K16[:], tc_[:], op=AX.add)

    # invariants: m2 = tr K (fp32), I3 = detH^2, I2 via Pool
    m2 = named("m2")
    nc.vector.tensor_tensor(m2[:], K16[:, 0, 0], K16[:, 1, 1], op=AX.add)
    nc.vector.tensor_tensor(m2[:], m2[:], K16[:, 2, 2], op=AX.add)
    I3 = named("I3")
    nc.vector.tensor_tensor(I3[:], detH[:], detH[:], op=AX.mult)

    o01 = ch("o01"); o02 = ch("o02"); o12 = ch("o12")
    nc.scalar.square(o01[:], K16[:, 0, 1])
    nc.scalar.square(o02[:], K16[:, 0, 2])
    nc.scalar.square(o12[:], K16[:, 1, 2])
    I2 = named("I2"); mm = ch("mm")
    nc.gpsimd.tensor_tensor(I2[:], K16[:, 0, 0], K16[:, 1, 1], op=AX.mult)
    nc.gpsimd.tensor_tensor(I2[:], I2[:], o01[:], op=AX.subtract)
    nc.gpsimd.tensor_tensor(mm[:], K16[:, 0, 0], K16[:, 2, 2], op=AX.mult)
    nc.gpsimd.tensor_tensor(mm[:], mm[:], o02[:], op=AX.subtract)
    nc.gpsimd.tensor_tensor(I2[:], I2[:], mm[:], op=AX.add)
    nc.gpsimd.tensor_tensor(mm[:], K16[:, 1, 1], K16[:, 2, 2], op=AX.mult)
    nc.gpsimd.tensor_tensor(mm[:], mm[:], o12[:], op=AX.subtract)
    nc.gpsimd.tensor_tensor(I2[:], I2[:], mm[:], op=AX.add)

    return {"H16": H16, "K16": K16, "detH": detH, "m2": m2, "I2": I2,
            "I3": I3, "o01": o01, "o02": o02, "o12": o12}


def _foam_chain(nc, sp_, chp, st, hf):
    """FOAM part A2: eigen chain (Cardano start, Newton, deflation), adjH."""
    fs = slice(hf * SF, hf * SF + SF)
    S3 = [P, 3, 3, SF]
    H16 = st["H16"]; K16 = st["K16"]; detH = st["detH"]
    m2 = st["m2"]; I2 = st["I2"]; I3 = st["I3"]
    o01 = st["o01"]; o02 = st["o02"]; o12 = st["o12"]

    def ch(name, dt=F32):
        return chp.tile([P, SF], dt, tag="ch32" if dt == F32 else "ch16",
                        name=name)

    def named(tag, dt=F32):
        return chp.tile([P, SF], dt, tag=tag, name=tag, bufs=1)

    # Cardano upper bound start: x0 = m2/3 + 2*sqrt((dsum + 2*osum)/6)
    q = named("q")
    nc.vector.tensor_scalar_mul(q[:], m2[:], 1.0 / 3.0)
    osum = ch("osum")
    nc.vector.tensor_tensor(osum[:], o01[:], o02[:], op=AX.add)
    nc.vector.tensor_tensor(osum[:], osum[:], o12[:], op=AX.add)
    dsum = ch("dsum"); kd = ch("kd"); kd2 = ch("kd2")
    nc.vector.tensor_tensor(kd[:], K16[:, 0, 0], q[:], op=AX.subtract)
    nc.vector.tensor_tensor(dsum[:], kd[:], kd[:], op=AX.mult)
    nc.vector.tensor_tensor(kd[:], K16[:, 1, 1], q[:], op=AX.subtract)
    nc.vector.tensor_tensor(kd2[:], kd[:], kd[:], op=AX.mult)
    nc.vector.tensor_tensor(dsum[:], dsum[:], kd2[:], op=AX.add)
    nc.vector.tensor_tensor(kd[:], K16[:, 2, 2], q[:], op=AX.subtract)
    nc.vector.tensor_tensor(kd2[:], kd[:], kd[:], op=AX.mult)
    nc.vector.tensor_tensor(dsum[:], dsum[:], kd2[:], op=AX.add)
    p2 = ch("p2")
    nc.vector.scalar_tensor_tensor(p2[:], osum[:], 2.0, dsum[:],
                                   op0=AX.mult, op1=AX.add)
    pC = ch("pC")
    nc.scalar.activation(pC[:], p2[:], AF.Sqrt, scale=1.0 / 6.0)
    X = named("X")
    nc.vector.scalar_tensor_tensor(X[:], pC[:], 2.0, q[:],
                                   op0=AX.mult, op1=AX.add)

    # Newton on f(x) = ((x - m2) x + I2) x - I3, 2 iters from above
    m2_2 = named("m2_2")
    nc.vector.tensor_scalar_mul(m2_2[:], m2[:], 2.0)
    na = ch("na"); nb = ch("nb")
    for _ in range(2):
        nc.vector.tensor_tensor(na[:], X[:], m2[:], op=AX.subtract)
        nc.vector.tensor_tensor(na[:], na[:], X[:], op=AX.mult)
        nc.vector.tensor_tensor(na[:], na[:], I2[:], op=AX.add)
        nc.vector.tensor_tensor(na[:], na[:], X[:], op=AX.mult)
        nc.vector.tensor_tensor(na[:], na[:], I3[:], op=AX.subtract)   # f
        nc.vector.tensor_scalar_mul(nb[:], X[:], 3.0)
        nc.vector.tensor_tensor(nb[:], nb[:], m2_2[:], op=AX.subtract)
        nc.vector.tensor_tensor(nb[:], nb[:], X[:], op=AX.mult)
        nc.vector.tensor_tensor(nb[:], nb[:], I2[:], op=AX.add)        # f'
        nc.vector.reciprocal(nb[:], nb[:])
        nc.vector.tensor_tensor(na[:], na[:], nb[:], op=AX.mult)
        nc.vector.tensor_tensor(X[:], X[:], na[:], op=AX.subtract)

    # deflate: mu2/mu3 from x^2 - (m2-mu1)x + I3/mu1
    mus = chp.tile([P, 3, SF], F32, tag="mus", name="mus", bufs=1)
    mu1 = mus[:, 0]; mu2 = mus[:, 1]; mu3 = mus[:, 2]
    nc.vector.tensor_scalar_max(mu1, X[:], 1e-25)
    b = ch("b"); cc = ch("cc"); rmu = ch("rmu")
    nc.vector.tensor_tensor(b[:], m2[:], mu1, op=AX.subtract)
    nc.vector.reciprocal(rmu[:], mu1)
    nc.vector.tensor_tensor(cc[:], I3[:], rmu[:], op=AX.mult)
    b2 = ch("b2")
    nc.vector.tensor_tensor(b2[:], b[:], b[:], op=AX.mult)
    disc2 = ch("disc2")
    nc.vector.scalar_tensor_tensor(disc2[:], cc[:], -4.0, b2[:],
                                   op0=AX.mult, op1=AX.add)
    nc.vector.tensor_scalar_max(disc2[:], disc2[:], 0.0)
    disc = ch("disc")
    nc.scalar.sqrt(disc[:], disc2[:])
    bh = ch("bh")
    nc.vector.tensor_scalar_mul(bh[:], b[:], 0.5)
    nc.vector.scalar_tensor_tensor(mu2, disc[:], 0.5, bh[:],
                                   op0=AX.mult, op1=AX.add)
    nc.vector.tensor_scalar_max(mu2, mu2, 0.0)
    nc.vector.tensor_tensor(mu3, b[:], mu2, op=AX.subtract)
    nc.vector.tensor_scalar_max(mu3, mu3, 0.0)

    rt = chp.tile([P, 3, SF], F32, tag="rt", name="rt", bufs=1)
    nc.scalar.sqrt(rt[:], mus[:])
    sgn = ch("sgn")
    nc.scalar.sign(sgn[:], detH[:])
    lam = named("lam")
    nc.vector.tensor_tensor(lam[:], rt[:, 0], rt[:, 1], op=AX.add)
    s3s = ch("s3s")
    nc.vector.tensor_tensor(s3s[:], sgn[:], rt[:, 2], op=AX.mult)
    nc.vector.tensor_tensor(lam[:], lam[:], s3s[:], op=AX.add)

    # alpha2 = lam^2 + m2 ; zeta2 = (lam^2 - m2) lam - 2 detH (floored)
    lam2 = ch("lam2"); alpha2 = named("alpha2")
    nc.vector.tensor_tensor(lam2[:], lam[:], lam[:], op=AX.mult)
    nc.vector.tensor_tensor(alpha2[:], lam2[:], m2[:], op=AX.add)
    zt = ch("zt")
    nc.vector.tensor_tensor(zt[:], lam2[:], m2[:], op=AX.subtract)
    nc.vector.tensor_tensor(zt[:], zt[:], lam[:], op=AX.mult)
    zeta2 = ch("zeta2")
    nc.vector.scalar_tensor_tensor(zeta2[:], detH[:], -2.0, zt[:],
                                   op0=AX.mult, op1=AX.add)
    m2s = ch("m2s")
    nc.scalar.sqrt(m2s[:], m2[:])
    zfl = ch("zfl")
    nc.vector.scalar_tensor_tensor(zfl[:], m2s[:], 1e-4, m2[:],
                                   op0=AX.mult, op1=AX.mult)
    nc.vector.tensor_tensor(zeta2[:], zeta2[:], zfl[:], op=AX.max)
    rz = ch("rz")
    nc.vector.reciprocal(rz[:], zeta2[:])

    # fp16 stage for the slab assembly
    a16 = named("a16", F16)
    nc.vector.tensor_copy(a16[:], alpha2[:])
    l16 = named("l16", F16)
    nc.vector.tensor_scalar_mul(l16[:], lam[:], 2.0)
    rz16 = named("rz16", F16)
    nc.vector.tensor_copy(rz16[:], rz[:])

    # adjugate of H: fp16 channel ops on DVE (cheap in 2x mode)
    adjH = sp_.tile(S3, F16, tag="adjH", bufs=1, name="adjH")
    idx = [
        (0, 0, (1, 1), (2, 2), (1, 2), (2, 1)),
        (0, 1, (0, 2), (2, 1), (0, 1), (2, 2)),
        (0, 2, (0, 1), (1, 2), (0, 2), (1, 1)),
        (1, 0, (1, 2), (2, 0), (1, 0), (2, 2)),
        (1, 1, (0, 0), (2, 2), (0, 2), (2, 0)),
        (1, 2, (0, 2), (1, 0), (0, 0), (1, 2)),
        (2, 0, (1, 0), (2, 1), (1, 1), (2, 0)),
        (2, 1, (0, 1), (2, 0), (0, 0), (2, 1)),
        (2, 2, (0, 0), (1, 1), (0, 1), (1, 0)),
    ]
    aw1 = ch("aw1", F16); aw2 = ch("aw2", F16)
    for (i, j, (a1, a2), (b1, b2), (c1_, c2_), (d1, d2)) in idx:
        nc.vector.tensor_tensor(aw1[:], H16[:, a1, a2], H16[:, b1, b2], op=AX.mult)
        nc.vector.tensor_tensor(aw2[:], H16[:, c1_, c2_], H16[:, d1, d2], op=AX.mult)
        nc.vector.tensor_tensor(adjH[:, i, j], aw1[:], aw2[:], op=AX.subtract)

    return {"H16": H16, "K16": K16, "adjH": adjH,
            "a16": a16, "l16": l16, "rz16": rz16}


def _foam_b(nc, sp_, chp, st, G16, SPh, STh, R16, V16, t16, hf):
    return _foam_half_b(nc, sp_, chp, st, G16, SPh, STh, R16, V16, t16, hf)


def _foam_half_b(nc, sp_, chp, st, G16, SPh, STh, R16, V16, t16, hf):
    """FOAM part B: slab assembly, R, V, and the V fold into t16."""
    fs = slice(hf * SF, hf * SF + SF)
    S3 = [P, 3, 3, SF]
    SPv = SPh[:, :, fs]
    STv = STh[:, :, fs]
    H16 = st["H16"]; K16 = st["K16"]; adjH = st["adjH"]
    a16 = st["a16"]; l16 = st["l16"]; rz16 = st["rz16"]

    def slab(name):
        return sp_.tile(S3, F16, tag="ktmp", name=name)

    # num = (alpha2 I - 2K) H^T + 2 lam adjH ;  R = num / zeta2, clamped
    W = sp_.tile(S3, F16, tag="Mt", bufs=1, name="W")
    nc.vector.tensor_scalar_mul(W[:], K16[:], -2.0)
    # diagonal view: stride 4*SF within the contiguous [3,3,SF] block
    nc.vector.tensor_tensor(
        W[:].rearrange("p a b s -> p (a b) s")[:, 0:9:4, :],
        W[:].rearrange("p a b s -> p (a b) s")[:, 0:9:4, :],
        a16[:].unsqueeze(1).broadcast_to([P, 3, SF]), op=AX.add)
    Ht = H16[:].transpose([0, 2, 1, 3])
    num = slab("num")
    nc.vector.tensor_tensor(num[:], W[:, :, 0].unsqueeze(2).broadcast_to(S3),
                            H16[:, :, 0].unsqueeze(1).broadcast_to(S3), op=AX.mult)
    for c in (1, 2):
        uc = slab(f"u{c}")
        nc.vector.tensor_tensor(uc[:], W[:, :, c].unsqueeze(2).broadcast_to(S3),
                                H16[:, :, c].unsqueeze(1).broadcast_to(S3),
                                op=AX.mult)
        nc.vector.tensor_tensor(num[:], num[:], uc[:], op=AX.add)
    vB = slab("vB")
    nc.vector.tensor_tensor(
        vB[:], l16[:].unsqueeze(1).unsqueeze(2).broadcast_to(S3), adjH[:],
        op=AX.mult)
    nc.vector.tensor_tensor(num[:], num[:], vB[:], op=AX.add)
    R16v = R16[:, :, :, fs]
    nc.vector.tensor_tensor(
        R16v, num[:], rz16[:].unsqueeze(1).unsqueeze(2).broadcast_to(S3),
        op=AX.mult)
    nc.vector.tensor_scalar(R16v, R16v, 4.0, -4.0, op0=AX.min, op1=AX.max)

    # V = (STh - R SPh) / sqrt(14)  (== t_mean - R p_mean)
    pv_ = slab("pv_")
    nc.vector.tensor_tensor(pv_[:], R16v, SPv.unsqueeze(1).broadcast_to(S3),
                            op=AX.mult)
    RS = chp.tile([P, 3, SF], F16, tag="RS", name="RS", bufs=1)
    nc.vector.tensor_tensor(RS[:], pv_[:, :, 0], pv_[:, :, 1], op=AX.add)
    nc.vector.tensor_tensor(RS[:], RS[:], pv_[:, :, 2], op=AX.add)
    Vt = chp.tile([P, 3, SF], F16, tag="Vt", name="Vt", bufs=1)
    nc.vector.tensor_tensor(Vt[:], STv, RS[:], op=AX.subtract)
    nc.vector.tensor_scalar_mul(V16[:, :, fs], Vt[:], SQ14I)

    # fold V into t16 in place: pass3's residual becomes qv - t16
    TSH = [P, 3, 14, SF]
    nc.vector.tensor_tensor(
        t16[:, :, :, fs], t16[:, :, :, fs],
        V16[:, :, fs].unsqueeze(2).broadcast_to(TSH), op=AX.subtract)


def _pass3_chunk(nc, workp, p16, t16, R16, V16, n2P, dvps, accP, I16, nI16,
                 ci, use_pe):
    cs = slice(ci * NB, (ci + 1) * NB)
    CS = [P, 3, 14, NB]
    # prq[k][i, j, s] = R_ik p_kj
    prqs = []
    for k in range(3):
        prq = workp.tile(CS, F16, tag=f"prq{k}", name=f"prq{k}")
        nc.vector.tensor_tensor(
            prq[:], R16[:, :, k, cs].unsqueeze(2).broadcast_to(CS),
            p16[:, k, :, cs].unsqueeze(1).broadcast_to(CS), op=AX.mult)
        prqs.append(prq)
    dv2 = workp.tile(CS, F16, tag="dv2", name="dv2")
    tcs = t16[:, :, :, cs]
    if use_pe:
        # PE sums over k and subtracts t16 (V already folded into t16)
        subs = [(i * 12, min(12, NB - i * 12)) for i in range((NB + 11) // 12)]
        for si, (s0, sw) in enumerate(subs):
            ss = slice(s0, s0 + sw)
            dvp = dvps[si % len(dvps)]
            for k in range(3):
                nc.tensor.matmul(dvp[:, :, :, 0:sw], I16[:],
                                 prqs[k][:, :, :, ss],
                                 start=(k == 0), stop=False)
            nc.tensor.matmul(dvp[:, :, :, 0:sw], nI16[:], tcs[:, :, :, ss],
                             start=False, stop=True)
            nc.scalar.square(dv2[:, :, :, ss], dvp[:, :, :, 0:sw])
    else:
        # DVE sums (tail chunks: PE is the critical engine there)
        nc.vector.tensor_tensor(prqs[0][:], prqs[0][:], prqs[1][:], op=AX.add)
        nc.vector.tensor_tensor(prqs[0][:], prqs[0][:], prqs[2][:], op=AX.add)
        nc.vector.tensor_tensor(prqs[0][:], prqs[0][:], tcs, op=AX.subtract)
        nc.scalar.square(dv2[:], prqs[0][:])
    for sub in range(2):
        ss = slice(sub * 32, sub * 32 + 32)
        ov = n2P[:][:, ss, 0:14].transpose([0, 2, 1])
        for c in range(3):
            nc.tensor.matmul(ov, I16[:], dv2[:, c, :, ss],
                             start=(c == 0), stop=(c == 2))
    scrP = workp.tile([P, 64, 14], F16, tag="scrP", name="scrP")
    nc.scalar.activation(scrP[:], n2P[:][:, :, 0:14], AF.Sqrt,
                         accum_out=accP[:, ci:ci + 1])


def build_bass():
    nc = bacc.Bacc("TRN2")
    pred = nc.dram_tensor("pred", [B_LOC, CJ], F32, kind="ExternalInput")
    targ = nc.dram_tensor("target", [B_LOC, CJ], F32, kind="ExternalInput")
    out = nc.dram_tensor("out", [P, 3 * NACC], F32, kind="ExternalOutput")

    pv = pred[:].rearrange("(p n) d -> p n d", p=P)   # [128, 512, 42]
    tv = targ[:].rearrange("(p n) d -> p n d", p=P)

    with tile.TileContext(nc) as tc:
        with tc.tile_pool(name="persist", bufs=1) as pp:
            p16 = pp.tile([P, 3, 14, S], F16, tag="p16")
            t16 = pp.tile([P, 3, 14, S], F16, tag="t16")
            G16 = pp.tile([P, 3, 3, S], F16, tag="G16")
            SPh = pp.tile([P, 3, S], F16, tag="SPh")
            STh = pp.tile([P, 3, S], F16, tag="STh")
            R16 = pp.tile([P, 3, 3, S], F16, tag="R16")
            V16 = pp.tile([P, 3, S], F16, tag="V16")
            accM = pp.tile([P, NACC], F32, tag="accM")
            accA = pp.tile([P, NACC], F32, tag="accA")
            accP = pp.tile([P, NACC], F32, tag="accP")
            I16 = pp.tile([P, P], F16, tag="I16")
            make_identity(nc, I16[:])
            nI16 = pp.tile([P, P], F16, tag="nI16")
            nc.vector.tensor_scalar_mul(nI16[:], I16[:], -1.0)

            # ---------------- pass 1 ----------------------------------------
            with tc.tile_pool(name="load1", bufs=2) as loadp, \
                 tc.tile_pool(name="work1", bufs=1) as workp, \
                 tc.tile_pool(name="ps1", bufs=1, space="PSUM") as psp:
                Gp2 = psp.tile([P, 2, 3, NB], F32, tag="Gp2")
                Gp1 = psp.tile([P, 1, 3, NB], F32, tag="Gp1")
                SPp = psp.tile([P, 3, NB], F32, tag="SPp")
                STp = psp.tile([P, 3, NB], F32, tag="STp")
                n2M = psp.tile([P, 64, 16], F32, tag="n2M", name="n2M")
                n2A = psp.tile([P, 64, 16], F32, tag="n2A", name="n2A")
                for ci in range(NCHUNK):
                    _pass1_chunk(nc, loadp, workp, pv, tv, p16, t16,
                                 Gp2, Gp1, SPp, STp, n2M, n2A,
                                 accM, accA, G16, SPh, STh, I16, ci)

            # ---------------- FOAM + pass 3, interleaved --------------------
            with tc.tile_pool(name="slab_a", bufs=2) as sp_a, \
                 tc.tile_pool(name="ch_a", bufs=14) as chp_a, \
                 tc.tile_pool(name="work3", bufs=1) as workp3, \
                 tc.tile_pool(name="ps3", bufs=1, space="PSUM") as psp3:
                n2P = psp3.tile([P, 64, 16], F32, tag="n2P", name="n2P")
                dvps = [psp3.tile([P, 3, 14, 12], F32, tag=f"dvp{s}",
                                  name=f"dvp{s}") for s in range(2)]
                def P3(ci, use_pe=True):
                    _pass3_chunk(nc, workp3, p16, t16, R16, V16, n2P, dvps,
                                 accP, I16, nI16, ci, use_pe)
                st0 = _foam_inv(nc, sp_a, chp_a, G16, SPh, STh, R16, V16, 0)
                st0 = dict(st0, **_foam_chain(nc, sp_a, chp_a, st0, 0))
                _foam_b(nc, sp_a, chp_a, st0, G16, SPh, STh, R16, V16, t16, 0)
                st1 = _foam_inv(nc, sp_a, chp_a, G16, SPh, STh, R16, V16, 1)
                P3(0)
                st1 = dict(st1, **_foam_chain(nc, sp_a, chp_a, st1, 1))
                P3(1)
                P3(2)
                _foam_b(nc, sp_a, chp_a, st1, G16, SPh, STh, R16, V16, t16, 1)
                P3(3)
                for ci in range(4, 7):
                    P3(ci)
                P3(7, False)

            stage = pp.tile([P, 3 * NACC], F32, tag="stage", name="stage")
            nc.gpsimd.tensor_copy(stage[:, 0:NACC], accM[:])
            nc.gpsimd.tensor_copy(stage[:, NACC:2 * NACC], accP[:])
            nc.gpsimd.tensor_copy(stage[:, 2 * NACC:3 * NACC], accA[:])
            nc.sync.dma_start(out[:], stage[:])

    nc.finalize()
    return nc


_NC = None


def kernel(pred: np.ndarray, target: np.ndarray) -> np.ndarray:
    global _NC
    if _NC is None:
        _NC = build_bass()

    pred = np.ascontiguousarray(pred, dtype=np.float32).reshape(B_FULL, CJ)
    target = np.ascontiguousarray(target, dtype=np.float32).reshape(B_FULL, CJ)

    in_maps = []
    for c in range(N_CORES):
        sl = slice(c * B_LOC, (c + 1) * B_LOC)
        in_maps.append({"pred": pred[sl], "target": target[sl]})

    res = run_bass_kernel_spmd(_NC, in_maps, core_ids=list(range(N_CORES)))
    mp = pa = ac = 0.0
    for r in res.results:
        o = r["out"].astype(np.float64)
        mp += o[:, 0:NACC].sum()
        pa += o[:, NACC:2 * NACC].sum()
        ac += o[:, 2 * NACC:3 * NACC].sum()
    inv = 1.0 / SCALE
    return np.array([mp / (B_FULL * 14) * inv,
                     pa / (B_FULL * 14) * inv,
                     ac / (B_FULL * 12) * inv], dtype=np.float32)


# revision 34
# speedup vs baseline: 1.0369x; 1.0123x over previous
"""PoseMetrics (mpjpe / pa_mpjpe / accel_error) Trainium2 Bass kernel.

Full inputs: pred/target [524288, 3, 14] fp32. Output: [3] fp32.

Strategy (pure data parallel, 8 cores x 65536 samples):
  - Layout: 128 partitions x 512 samples/partition, samples innermost so the
    bulk fp16 elementwise work hits the DVE 2x mode. Inputs are converted
    once to persistent fp16 SBUF tiles (with a global 1/sqrt(8) prescale) and
    never re-streamed.
  - The tensor engine (PE) acts as a free accumulator: identity-weight
    matmuls into PSUM replace the j-sum trees (cross-covariance G, joint sums
    SP/ST) and the 3-way coordinate sums for the per-joint norms.
  - Kabsch/SVD is replaced by a closed form: K = H^T H, largest eigenvalue
    via cubic Newton (Cardano-bound start, 2 iters), remaining eigenvalues by
    quadratic deflation, lambda = s1+s2+sign(det H)*s3, then Markley's FOAM
    formula for R. Slab math fp16, eigen chain fp32.
  - Each core returns [128, 48] partial sums; host reduces in float64.
"""

import numpy as np

import concourse.bass as bass
import concourse.bacc as bacc
import concourse.mybir as mybir
import concourse.tile as tile
from concourse.bass_utils import run_bass_kernel_spmd
from concourse.masks import make_identity

F32 = mybir.dt.float32
F16 = mybir.dt.float16
AX = mybir.AluOpType
AF = mybir.ActivationFunctionType

N_CORES = 8
B_FULL = 524288
B_LOC = B_FULL // N_CORES          # 65536
P = 128                            # partitions
S = B_LOC // P                     # 512 samples per partition
NB = 64                            # samples per chunk (per partition)
NCHUNK = S // NB                   # 8
CJ = 42                            # 3*14
SF = 256                           # FOAM half size
SCALE = float(1.0 / np.sqrt(8.0))  # global input prescale (folded out on host)
SQ14I = float(1.0 / np.sqrt(14.0))
NACC = NCHUNK                      # accum slots per metric (1 per chunk)


def _pass1_chunk(nc, loadp, workp, pv, tv, p16, t16, Gp2, Gp1, SPp, STp,
                 n2M, n2A, accM, accA, G16, SPh, STh, I16, ci):
    cs = slice(ci * NB, (ci + 1) * NB)
    x32p = loadp.tile([P, NB, CJ], F32, tag="p32", name="x32p")
    x32t = loadp.tile([P, NB, CJ], F32, tag="t32", name="x32t")
    # fp32 -> fp16 J-major convert with the global prescale folded in.
    # On Pool: ACT is the pass-1 critical engine, Pool is idle here.
    # Chunk 0 is split into quarters so compute starts ~6us earlier.
    nsub = 4 if ci == 0 else 1
    sw = NB // nsub
    for si in range(nsub):
        ls = slice(si * sw, si * sw + sw)
        gs = slice(ci * NB + si * sw, ci * NB + si * sw + sw)
        nc.sync.dma_start(x32p[:, ls, :], pv[:, gs, :])
        nc.sync.dma_start(x32t[:, ls, :], tv[:, gs, :])
        nc.gpsimd.tensor_scalar_mul(
            p16[:, :, :, gs],
            x32p[:, ls, :].rearrange("p s (c j) -> p c j s", c=3, j=14), SCALE)
        nc.gpsimd.tensor_scalar_mul(
            t16[:, :, :, gs],
            x32t[:, ls, :].rearrange("p s (c j) -> p c j s", c=3, j=14), SCALE)

    pcs = p16[:, :, :, cs]
    tcs = t16[:, :, :, cs]

    # ---- mpjpe: d, d^2, PE c-sum, sqrt-accum --------------------------------
    d = workp.tile([P, 3, 14, NB], F16, tag="d", name="d")
    nc.vector.tensor_tensor(d[:], pcs, tcs, op=AX.subtract)
    d2 = workp.tile([P, 3, 14, NB], F16, tag="d2", name="d2")
    nc.scalar.square(d2[:], d[:])
    for sub in range(2):
        ss = slice(sub * 32, sub * 32 + 32)
        ov = n2M[:][:, ss, 0:14].transpose([0, 2, 1])
        for c in range(3):
            nc.tensor.matmul(ov, I16[:], d2[:, c, :, ss],
                             start=(c == 0), stop=(c == 2))
    scrM = workp.tile([P, 64, 14], F16, tag="scrM", name="scrM")
    nc.scalar.activation(scrM[:], n2M[:][:, :, 0:14], AF.Sqrt,
                         accum_out=accM[:, ci:ci + 1])

    # ---- accel: second difference over j, squares, PE c-sum ----------------
    ta = workp.tile([P, 3, 12, NB], F16, tag="ta", name="ta")
    nc.vector.tensor_scalar_mul(ta[:], pcs[:, :, 1:13, :], -2.0)
    nc.vector.tensor_tensor(ta[:], ta[:], pcs[:, :, 0:12, :], op=AX.add)
    nc.vector.tensor_tensor(ta[:], ta[:], pcs[:, :, 2:14, :], op=AX.add)
    a2 = workp.tile([P, 3, 12, NB], F16, tag="a2", name="a2")
    nc.scalar.square(a2[:], ta[:])
    for sub in range(2):
        ss = slice(sub * 32, sub * 32 + 32)
        ov = n2A[:][:, ss, 0:12].transpose([0, 2, 1])
        for c in range(3):
            nc.tensor.matmul(ov, I16[:], a2[:, c, :, ss],
                             start=(c == 0), stop=(c == 2))
    scrA = workp.tile([P, 64, 12], F16, tag="scrA", name="scrA")
    nc.scalar.activation(scrA[:], n2A[:][:, :, 0:12], AF.Sqrt,
                         accum_out=accA[:, ci:ci + 1])

    # ---- G / SP / ST via PE -------------------------------------------------
    # prod[k, i, j, s] = p_i t_k; one TT per k keeps APs within 3 free dims.
    CS = [P, 3, 14, NB]
    prod = workp.tile([P, 3, 3, 14, NB], F16, tag="prod", name="prod")
    for k in range(3):
        nc.vector.tensor_tensor(
            prod[:, k], pcs,
            tcs[:, k].unsqueeze(1).broadcast_to(CS), op=AX.mult)
    # G16[k, i] = sum_j prod[k, i, j]; split k to fit PSUM banks
    for (gp, ksl, nk) in ((Gp2, slice(0, 2), 2), (Gp1, slice(2, 3), 1)):
        for j in range(14):
            nc.tensor.matmul(gp[:], I16[:], prod[:, ksl, :, j, :],
                             start=(j == 0), stop=(j == 13))
    for j in range(14):
        nc.tensor.matmul(SPp[:], I16[:], p16[:, :, j, cs],
                         start=(j == 0), stop=(j == 13))
    for j in range(14):
        nc.tensor.matmul(STp[:], I16[:], t16[:, :, j, cs],
                         start=(j == 0), stop=(j == 13))

    # drains: G + SP/ST on ACT (GPSIMD cannot read PSUM)
    nc.scalar.copy(G16[:, 0:2, :, cs], Gp2[:])
    nc.scalar.copy(G16[:, 2:3, :, cs], Gp1[:])
    nc.scalar.activation(SPh[:, :, cs], SPp[:], AF.Copy, scale=SQ14I)
    nc.scalar.activation(STh[:, :, cs], STp[:], AF.Copy, scale=SQ14I)


def _foam_inv(nc, sp_, chp, G16, SPh, STh, R16, V16, hf):
    """FOAM part A1: H, K, det, invariants.

    H is in s^2 = 1/8 scale (inherited from the input prescale); the FOAM
    formula is scale-invariant so no rescaling is needed anywhere.
    SPh/STh are joint sums scaled by 1/sqrt(14).
    """
    fs = slice(hf * SF, hf * SF + SF)
    S3 = [P, 3, 3, SF]
    # G16 is stored (k, i); present it as (i, k) via a stride view
    Gv = G16[:, :, :, fs].transpose([0, 2, 1, 3])
    SPv = SPh[:, :, fs]
    STv = STh[:, :, fs]

    def slab(name):
        # rotating scratch slab; at most `bufs` of these live at once
        return sp_.tile(S3, F16, tag="ktmp", name=name)

    def ch(name, dt=F32):
        return chp.tile([P, SF], dt, tag="ch32" if dt == F32 else "ch16",
                        name=name)

    def named(tag, dt=F32):
        return chp.tile([P, SF], dt, tag=tag, name=tag, bufs=1)

    # H = G - SP ST^T / 14  (SPh*STh = SP*ST/14 already)
    outer = slab("outer")
    nc.vector.tensor_tensor(
        outer[:], SPv.unsqueeze(2).broadcast_to(S3),
        STv.unsqueeze(1).broadcast_to(S3), op=AX.mult)
    H16 = sp_.tile(S3, F16, tag="H16", bufs=1, name="H16")
    nc.vector.tensor_tensor(H16[:], Gv, outer[:], op=AX.subtract)

    # detH on Pool (fp32 out), from fp16 H
    detH = named("detH")
    c1 = ch("det_c1"); c2 = ch("det_c2"); acc = ch("det_acc")
    nc.gpsimd.tensor_tensor(c1[:], H16[:, 1, 1], H16[:, 2, 2], op=AX.mult)
    nc.gpsimd.tensor_tensor(c2[:], H16[:, 1, 2], H16[:, 2, 1], op=AX.mult)
    nc.gpsimd.tensor_tensor(c1[:], c1[:], c2[:], op=AX.subtract)
    nc.gpsimd.tensor_tensor(acc[:], H16[:, 0, 0], c1[:], op=AX.mult)
    nc.gpsimd.tensor_tensor(c1[:], H16[:, 1, 0], H16[:, 2, 2], op=AX.mult)
    nc.gpsimd.tensor_tensor(c2[:], H16[:, 1, 2], H16[:, 2, 0], op=AX.mult)
    nc.gpsimd.tensor_tensor(c1[:], c1[:], c2[:], op=AX.subtract)
    nc.gpsimd.tensor_tensor(c1[:], H16[:, 0, 1], c1[:], op=AX.mult)
    nc.gpsimd.tensor_tensor(acc[:], acc[:], c1[:], op=AX.subtract)
    nc.gpsimd.tensor_tensor(c1[:], H16[:, 1, 0], H16[:, 2, 1], op=AX.mult)
    nc.gpsimd.tensor_tensor(c2[:], H16[:, 1, 1], H16[:, 2, 0], op=AX.mult)
    nc.gpsimd.tensor_tensor(c1[:], c1[:], c2[:], op=AX.subtract)
    nc.gpsimd.tensor_tensor(c1[:], H16[:, 0, 2], c1[:], op=AX.mult)
    nc.gpsimd.tensor_tensor(detH[:], acc[:], c1[:], op=AX.add)

    # K = H^T H (fp16 slabs, accumulate into K16)
    K16 = sp_.tile(S3, F16, tag="K16", bufs=1, name="K16")
    nc.vector.tensor_tensor(K16[:], H16[:, 0].unsqueeze(2).broadcast_to(S3),
                            H16[:, 0].unsqueeze(1).broadcast_to(S3), op=AX.mult)
    for c in (1, 2):
        tc_ = slab(f"t{c}")
        nc.vector.tensor_tensor(tc_[:], H16[:, c].unsqueeze(2).broadcast_to(S3),
                                H16[:, c].unsqueeze(1).broadcast_to(S3),
                                op=AX.mult)
        nc.vector.tensor_tensor(K16[:], K16[:], tc_[:], op=AX.add)

    # invariants: m2 = tr K (fp32), I3 = detH^2, I2 via Pool
    m2 = named("m2")
    nc.vector.tensor_tensor(m2[:], K16[:, 0, 0], K16[:, 1, 1], op=AX.add)
    nc.vector.tensor_tensor(m2[:], m2[:], K16[:, 2, 2], op=AX.add)
    I3 = named("I3")
    nc.vector.tensor_tensor(I3[:], detH[:], detH[:], op=AX.mult)

    o01 = ch("o01"); o02 = ch("o02"); o12 = ch("o12")
    nc.scalar.square(o01[:], K16[:, 0, 1])
    nc.scalar.square(o02[:], K16[:, 0, 2])
    nc.scalar.square(o12[:], K16[:, 1, 2])
    I2 = named("I2"); mm = ch("mm")
    nc.gpsimd.tensor_tensor(I2[:], K16[:, 0, 0], K16[:, 1, 1], op=AX.mult)
    nc.gpsimd.tensor_tensor(I2[:], I2[:], o01[:], op=AX.subtract)
    nc.gpsimd.tensor_tensor(mm[:], K16[:, 0, 0], K16[:, 2, 2], op=AX.mult)
    nc.gpsimd.tensor_tensor(mm[:], mm[:], o02[:], op=AX.subtract)
    nc.gpsimd.tensor_tensor(I2[:], I2[:], mm[:], op=AX.add)
    nc.gpsimd.tensor_tensor(mm[:], K16[:, 1, 1], K16[:, 2, 2], op=AX.mult)
    nc.gpsimd.tensor_tensor(mm[:], mm[:], o12[:], op=AX.subtract)
    nc.gpsimd.tensor_tensor(I2[:], I2[:], mm[:], op=AX.add)

    return {"H16": H16, "K16": K16, "detH": detH, "m2": m2, "I2": I2,
            "I3": I3, "o01": o01, "o02": o02, "o12": o12}


def _foam_chain(nc, sp_, chp, st, hf):
    """FOAM part A2: eigen chain (Cardano start, Newton, deflation), adjH."""
    fs = slice(hf * SF, hf * SF + SF)
    S3 = [P, 3, 3, SF]
    H16 = st["H16"]; K16 = st["K16"]; detH = st["detH"]
    m2 = st["m2"]; I2 = st["I2"]; I3 = st["I3"]
    o01 = st["o01"]; o02 = st["o02"]; o12 = st["o12"]

    def ch(name, dt=F32):
        return chp.tile([P, SF], dt, tag="ch32" if dt == F32 else "ch16",
                        name=name)

    def named(tag, dt=F32):
        return chp.tile([P, SF], dt, tag=tag, name=tag, bufs=1)

    # Cardano upper bound start: x0 = m2/3 + 2*sqrt((dsum + 2*osum)/6)
    q = named("q")
    nc.vector.tensor_scalar_mul(q[:], m2[:], 1.0 / 3.0)
    osum = ch("osum")
    nc.vector.tensor_tensor(osum[:], o01[:], o02[:], op=AX.add)
    nc.vector.tensor_tensor(osum[:], osum[:], o12[:], op=AX.add)
    dsum = ch("dsum"); kd = ch("kd"); kd2 = ch("kd2")
    nc.vector.tensor_tensor(kd[:], K16[:, 0, 0], q[:], op=AX.subtract)
    nc.vector.tensor_tensor(dsum[:], kd[:], kd[:], op=AX.mult)
    nc.vector.tensor_tensor(kd[:], K16[:, 1, 1], q[:], op=AX.subtract)
    nc.vector.tensor_tensor(kd2[:], kd[:], kd[:], op=AX.mult)
    nc.vector.tensor_tensor(dsum[:], dsum[:], kd2[:], op=AX.add)
    nc.vector.tensor_tensor(kd[:], K16[:, 2, 2], q[:], op=AX.subtract)
    nc.vector.tensor_tensor(kd2[:], kd[:], kd[:], op=AX.mult)
    nc.vector.tensor_tensor(dsum[:], dsum[:], kd2[:], op=AX.add)
    p2 = ch("p2")
    nc.vector.scalar_tensor_tensor(p2[:], osum[:], 2.0, dsum[:],
                                   op0=AX.mult, op1=AX.add)
    pC = ch("pC")
    nc.scalar.activation(pC[:], p2[:], AF.Sqrt, scale=1.0 / 6.0)
    X = named("X")
    nc.vector.scalar_tensor_tensor(X[:], pC[:], 2.0, q[:],
                                   op0=AX.mult, op1=AX.add)

    # Newton on f(x) = ((x - m2) x + I2) x - I3, 2 iters from above
    m2_2 = named("m2_2")
    nc.vector.tensor_scalar_mul(m2_2[:], m2[:], 2.0)
    na = ch("na"); nb = ch("nb")
    for _ in range(2):
        nc.vector.tensor_tensor(na[:], X[:], m2[:], op=AX.subtract)
        nc.vector.tensor_tensor(na[:], na[:], X[:], op=AX.mult)
        nc.vector.tensor_tensor(na[:], na[:], I2[:], op=AX.add)
        nc.vector.tensor_tensor(na[:], na[:], X[:], op=AX.mult)
        nc.vector.tensor_tensor(na[:], na[:], I3[:], op=AX.subtract)   # f
        nc.vector.tensor_scalar_mul(nb[:], X[:], 3.0)
        nc.vector.tensor_tensor(nb[:], nb[:], m2_2[:], op=AX.subtract)
        nc.vector.tensor_tensor(nb[:], nb[:], X[:], op=AX.mult)
        nc.vector.tensor_tensor(nb[:], nb[:], I2[:], op=AX.add)        # f'
        nc.vector.reciprocal(nb[:], nb[:])
        nc.vector.tensor_tensor(na[:], na[:], nb[:], op=AX.mult)
        nc.vector.tensor_tensor(X[:], X[:], na[:], op=AX.subtract)

    # deflate: mu2/mu3 from x^2 - (m2-mu1)x + I3/mu1
    mus = chp.tile([P, 3, SF], F32, tag="mus", name="mus", bufs=1)
    mu1 = mus[:, 0]; mu2 = mus[:, 1]; mu3 = mus[:, 2]
    nc.vector.tensor_scalar_max(mu1, X[:], 1e-25)
    b = ch("b"); cc = ch("cc"); rmu = ch("rmu")
    nc.vector.tensor_tensor(b[:], m2[:], mu1, op=AX.subtract)
    nc.vector.reciprocal(rmu[:], mu1)
    nc.vector.tensor_tensor(cc[:], I3[:], rmu[:], op=AX.mult)
    b2 = ch("b2")
    nc.vector.tensor_tensor(b2[:], b[:], b[:], op=AX.mult)
    disc2 = ch("disc2")
    nc.vector.scalar_tensor_tensor(disc2[:], cc[:], -4.0, b2[:],
                                   op0=AX.mult, op1=AX.add)
    nc.vector.tensor_scalar_max(disc2[:], disc2[:], 0.0)
    disc = ch("disc")
    nc.scalar.sqrt(disc[:], disc2[:])
    bh = ch("bh")
    nc.vector.tensor_scalar_mul(bh[:], b[:], 0.5)
    nc.vector.scalar_tensor_tensor(mu2, disc[:], 0.5, bh[:],
                                   op0=AX.mult, op1=AX.add)
    nc.vector.tensor_scalar_max(mu2, mu2, 0.0)
    nc.vector.tensor_tensor(mu3, b[:], mu2, op=AX.subtract)
    nc.vector.tensor_scalar_max(mu3, mu3, 0.0)

    rt = chp.tile([P, 3, SF], F32, tag="rt", name="rt", bufs=1)
    nc.scalar.sqrt(rt[:], mus[:])
    sgn = ch("sgn")
    nc.scalar.sign(sgn[:], detH[:])
    lam = named("lam")
    nc.vector.tensor_tensor(lam[:], rt[:, 0], rt[:, 1], op=AX.add)
    s3s = ch("s3s")
    nc.vector.tensor_tensor(s3s[:], sgn[:], rt[:, 2], op=AX.mult)
    nc.vector.tensor_tensor(lam[:], lam[:], s3s[:], op=AX.add)

    # alpha2 = lam^2 + m2 ; zeta2 = (lam^2 - m2) lam - 2 detH (floored)
    lam2 = ch("lam2"); alpha2 = named("alpha2")
    nc.vector.tensor_tensor(lam2[:], lam[:], lam[:], op=AX.mult)
    nc.vector.tensor_tensor(alpha2[:], lam2[:], m2[:], op=AX.add)
    zt = ch("zt")
    nc.vector.tensor_tensor(zt[:], lam2[:], m2[:], op=AX.subtract)
    nc.vector.tensor_tensor(zt[:], zt[:], lam[:], op=AX.mult)
    zeta2 = ch("zeta2")
    nc.vector.scalar_tensor_tensor(zeta2[:], detH[:], -2.0, zt[:],
                                   op0=AX.mult, op1=AX.add)
    m2s = ch("m2s")
    nc.scalar.sqrt(m2s[:], m2[:])
    zfl = ch("zfl")
    nc.vector.scalar_tensor_tensor(zfl[:], m2s[:], 1e-4, m2[:],
                                   op0=AX.mult, op1=AX.mult)
    nc.vector.tensor_tensor(zeta2[:], zeta2[:], zfl[:], op=AX.max)
    rz = ch("rz")
    nc.vector.reciprocal(rz[:], zeta2[:])

    # fp16 stage for the slab assembly
    a16 = named("a16", F16)
    nc.vector.tensor_copy(a16[:], alpha2[:])
    l16 = named("l16", F16)
    nc.vector.tensor_scalar_mul(l16[:], lam[:], 2.0)
    rz16 = named("rz16", F16)
    nc.vector.tensor_copy(rz16[:], rz[:])

    # adjugate of H: fp16 channel ops on DVE (cheap in 2x mode)
    adjH = sp_.tile(S3, F16, tag="adjH", bufs=1, name="adjH")
    idx = [
        (0, 0, (1, 1), (2, 2), (1, 2), (2, 1)),
        (0, 1, (0, 2), (2, 1), (0, 1), (2, 2)),
        (0, 2, (0, 1), (1, 2), (0, 2), (1, 1)),
        (1, 0, (1, 2), (2, 0), (1, 0), (2, 2)),
        (1, 1, (0, 0), (2, 2), (0, 2), (2, 0)),
        (1, 2, (0, 2), (1, 0), (0, 0), (1, 2)),
        (2, 0, (1, 0), (2, 1), (1, 1), (2, 0)),
        (2, 1, (0, 1), (2, 0), (0, 0), (2, 1)),
        (2, 2, (0, 0), (1, 1), (0, 1), (1, 0)),
    ]
    aw1 = ch("aw1", F16); aw2 = ch("aw2", F16)
    for (i, j, (a1, a2), (b1, b2), (c1_, c2_), (d1, d2)) in idx:
        nc.vector.tensor_tensor(aw1[:], H16[:, a1, a2], H16[:, b1, b2], op=AX.mult)
        nc.vector.tensor_tensor(aw2[:], H16[:, c1_, c2_], H16[:, d1, d2], op=AX.mult)
        nc.vector.tensor_tensor(adjH[:, i, j], aw1[:], aw2[:], op=AX.subtract)

    return {"H16": H16, "K16": K16, "adjH": adjH,
            "a16": a16, "l16": l16, "rz16": rz16}


def _foam_b(nc, sp_, chp, st, G16, SPh, STh, R16, V16, t16, hf):
    return _foam_half_b(nc, sp_, chp, st, G16, SPh, STh, R16, V16, t16, hf)


def _foam_half_b(nc, sp_, chp, st, G16, SPh, STh, R16, V16, t16, hf):
    """FOAM part B: slab assembly, R, V, and the V fold into t16."""
    fs = slice(hf * SF, hf * SF + SF)
    S3 = [P, 3, 3, SF]
    SPv = SPh[:, :, fs]
    STv = STh[:, :, fs]
    H16 = st["H16"]; K16 = st["K16"]; adjH = st["adjH"]
    a16 = st["a16"]; l16 = st["l16"]; rz16 = st["rz16"]

    def slab(name):
        return sp_.tile(S3, F16, tag="ktmp", name=name)

    # num = (alpha2 I - 2K) H^T + 2 lam adjH ;  R = num / zeta2, clamped
    W = sp_.tile(S3, F16, tag="Mt", bufs=1, name="W")
    nc.vector.tensor_scalar_mul(W[:], K16[:], -2.0)
    # diagonal view: stride 4*SF within the contiguous [3,3,SF] block
    nc.vector.tensor_tensor(
        W[:].rearrange("p a b s -> p (a b) s")[:, 0:9:4, :],
        W[:].rearrange("p a b s -> p (a b) s")[:, 0:9:4, :],
        a16[:].unsqueeze(1).broadcast_to([P, 3, SF]), op=AX.add)
    Ht = H16[:].transpose([0, 2, 1, 3])
    num = slab("num")
    nc.vector.tensor_tensor(num[:], W[:, :, 0].unsqueeze(2).broadcast_to(S3),
                            H16[:, :, 0].unsqueeze(1).broadcast_to(S3), op=AX.mult)
    for c in (1, 2):
        uc = slab(f"u{c}")
        nc.vector.tensor_tensor(uc[:], W[:, :, c].unsqueeze(2).broadcast_to(S3),
                                H16[:, :, c].unsqueeze(1).broadcast_to(S3),
                                op=AX.mult)
        nc.vector.tensor_tensor(num[:], num[:], uc[:], op=AX.add)
    vB = slab("vB")
    nc.vector.tensor_tensor(
        vB[:], l16[:].unsqueeze(1).unsqueeze(2).broadcast_to(S3), adjH[:],
        op=AX.mult)
    nc.vector.tensor_tensor(num[:], num[:], vB[:], op=AX.add)
    R16v = R16[:, :, :, fs]
    nc.vector.tensor_tensor(
        R16v, num[:], rz16[:].unsqueeze(1).unsqueeze(2).broadcast_to(S3),
        op=AX.mult)
    nc.vector.tensor_scalar(R16v, R16v, 4.0, -4.0, op0=AX.min, op1=AX.max)

    # V = (STh - R SPh) / sqrt(14)  (== t_mean - R p_mean)
    pv_ = slab("pv_")
    nc.vector.tensor_tensor(pv_[:], R16v, SPv.unsqueeze(1).broadcast_to(S3),
                            op=AX.mult)
    RS = chp.tile([P, 3, SF], F16, tag="RS", name="RS", bufs=1)
    nc.vector.tensor_tensor(RS[:], pv_[:, :, 0], pv_[:, :, 1], op=AX.add)
    nc.vector.tensor_tensor(RS[:], RS[:], pv_[:, :, 2], op=AX.add)
    Vt = chp.tile([P, 3, SF], F16, tag="Vt", name="Vt", bufs=1)
    nc.vector.tensor_tensor(Vt[:], STv, RS[:], op=AX.subtract)
    nc.vector.tensor_scalar_mul(V16[:, :, fs], Vt[:], SQ14I)

    # fold V into t16 in place: pass3's residual becomes qv - t16
    TSH = [P, 3, 14, SF]
    nc.vector.tensor_tensor(
        t16[:, :, :, fs], t16[:, :, :, fs],
        V16[:, :, fs].unsqueeze(2).broadcast_to(TSH), op=AX.subtract)


def _pass3_chunk(nc, workp, p16, t16, R16, V16, n2P, dvps, accP, I16, nI16,
                 ci, use_pe):
    cs = slice(ci * NB, (ci + 1) * NB)
    CS = [P, 3, 14, NB]
    # prq[k][i, j, s] = R_ik p_kj
    prqs = []
    for k in range(3):
        prq = workp.tile(CS, F16, tag=f"prq{k}", name=f"prq{k}")
        nc.vector.tensor_tensor(
            prq[:], R16[:, :, k, cs].unsqueeze(2).broadcast_to(CS),
            p16[:, k, :, cs].unsqueeze(1).broadcast_to(CS), op=AX.mult)
        prqs.append(prq)
    dv2 = workp.tile(CS, F16, tag="dv2", name="dv2")
    tcs = t16[:, :, :, cs]
    if use_pe:
        # PE sums over k and subtracts t16 (V already folded into t16)
        subs = [(i * 12, min(12, NB - i * 12)) for i in range((NB + 11) // 12)]
        for si, (s0, sw) in enumerate(subs):
            ss = slice(s0, s0 + sw)
            dvp = dvps[si % len(dvps)]
            for k in range(3):
                nc.tensor.matmul(dvp[:, :, :, 0:sw], I16[:],
                                 prqs[k][:, :, :, ss],
                                 start=(k == 0), stop=False)
            nc.tensor.matmul(dvp[:, :, :, 0:sw], nI16[:], tcs[:, :, :, ss],
                             start=False, stop=True)
            nc.scalar.square(dv2[:, :, :, ss], dvp[:, :, :, 0:sw])
    else:
        # DVE sums (tail chunks: PE is the critical engine there)
        nc.vector.tensor_tensor(prqs[0][:], prqs[0][:], prqs[1][:], op=AX.add)
        nc.vector.tensor_tensor(prqs[0][:], prqs[0][:], prqs[2][:], op=AX.add)
        nc.vector.tensor_tensor(prqs[0][:], prqs[0][:], tcs, op=AX.subtract)
        nc.scalar.square(dv2[:], prqs[0][:])
    for sub in range(2):
        ss = slice(sub * 32, sub * 32 + 32)
        ov = n2P[:][:, ss, 0:14].transpose([0, 2, 1])
        for c in range(3):
            nc.tensor.matmul(ov, I16[:], dv2[:, c, :, ss],
                             start=(c == 0), stop=(c == 2))
    scrP = workp.tile([P, 64, 14], F16, tag="scrP", name="scrP")
    nc.scalar.activation(scrP[:], n2P[:][:, :, 0:14], AF.Sqrt,
                         accum_out=accP[:, ci:ci + 1])


def build_bass():
    nc = bacc.Bacc("TRN2")
    pred = nc.dram_tensor("pred", [B_LOC, CJ], F32, kind="ExternalInput")
    targ = nc.dram_tensor("target", [B_LOC, CJ], F32, kind="ExternalInput")
    out = nc.dram_tensor("out", [P, 3 * NACC], F32, kind="ExternalOutput")

    pv = pred[:].rearrange("(p n) d -> p n d", p=P)   # [128, 512, 42]
    tv = targ[:].rearrange("(p n) d -> p n d", p=P)

    with tile.TileContext(nc) as tc:
        with tc.tile_pool(name="persist", bufs=1) as pp:
            p16 = pp.tile([P, 3, 14, S], F16, tag="p16")
            t16 = pp.tile([P, 3, 14, S], F16, tag="t16")
            G16 = pp.tile([P, 3, 3, S], F16, tag="G16")
            SPh = pp.tile([P, 3, S], F16, tag="SPh")
            STh = pp.tile([P, 3, S], F16, tag="STh")
            R16 = pp.tile([P, 3, 3, S], F16, tag="R16")
            V16 = pp.tile([P, 3, S], F16, tag="V16")
            accM = pp.tile([P, NACC], F32, tag="accM")
            accA = pp.tile([P, NACC], F32, tag="accA")
            accP = pp.tile([P, NACC], F32, tag="accP")
            I16 = pp.tile([P, P], F16, tag="I16")
            make_identity(nc, I16[:])
            nI16 = pp.tile([P, P], F16, tag="nI16")
            nc.vector.tensor_scalar_mul(nI16[:], I16[:], -1.0)

            # ---------------- pass 1 ----------------------------------------
            with tc.tile_pool(name="load1", bufs=2) as loadp, \
                 tc.tile_pool(name="work1", bufs=1) as workp, \
                 tc.tile_pool(name="ps1", bufs=1, space="PSUM") as psp:
                Gp2 = psp.tile([P, 2, 3, NB], F32, tag="Gp2")
                Gp1 = psp.tile([P, 1, 3, NB], F32, tag="Gp1")
                SPp = psp.tile([P, 3, NB], F32, tag="SPp")
                STp = psp.tile([P, 3, NB], F32, tag="STp")
                n2M = psp.tile([P, 64, 16], F32, tag="n2M", name="n2M")
                n2A = psp.tile([P, 64, 16], F32, tag="n2A", name="n2A")
                for ci in range(NCHUNK):
                    _pass1_chunk(nc, loadp, workp, pv, tv, p16, t16,
                                 Gp2, Gp1, SPp, STp, n2M, n2A,
                                 accM, accA, G16, SPh, STh, I16, ci)

            # ---------------- FOAM + pass 3, interleaved --------------------
            with tc.tile_pool(name="slab_a", bufs=2) as sp_a, \
                 tc.tile_pool(name="ch_a", bufs=14) as chp_a, \
                 tc.tile_pool(name="work3", bufs=1) as workp3, \
                 tc.tile_pool(name="ps3", bufs=1, space="PSUM") as psp3:
                n2P = psp3.tile([P, 64, 16], F32, tag="n2P", name="n2P")
                dvps = [psp3.tile([P, 3, 14, 12], F32, tag=f"dvp{s}",
                                  name=f"dvp{s}") for s in range(2)]
                def P3(ci, use_pe=True):
                    _pass3_chunk(nc, workp3, p16, t16, R16, V16, n2P, dvps,
                                 accP, I16, nI16, ci, use_pe)
                st0 = _foam_inv(nc, sp_a, chp_a, G16, SPh, STh, R16, V16, 0)
                st0 = dict(st0, **_foam_chain(nc, sp_a, chp_a, st0, 0))
                _foam_b(nc, sp_a, chp_a, st0, G16, SPh, STh, R16, V16, t16, 0)
                st1 = _foam_inv(nc, sp_a, chp_a, G16, SPh, STh, R16, V16, 1)
                st1 = dict(st1, **_foam_chain(nc, sp_a, chp_a, st1, 1))
                for ci in range(4):
                    P3(ci)
                _foam_b(nc, sp_a, chp_a, st1, G16, SPh, STh, R16, V16, t16, 1)
                for ci in range(4, NCHUNK):
                    P3(ci)

            stage = pp.tile([P, 3 * NACC], F32, tag="stage", name="stage")
            nc.gpsimd.tensor_copy(stage[:, 0:NACC], accM[:])
            nc.gpsimd.tensor_copy(stage[:, NACC:2 * NACC], accP[:])
            nc.gpsimd.tensor_copy(stage[:, 2 * NACC:3 * NACC], accA[:])
            nc.sync.dma_start(out[:], stage[:])

    nc.finalize()
    return nc


_NC = None


def kernel(pred: np.ndarray, target: np.ndarray) -> np.ndarray:
    global _NC
    if _NC is None:
        _NC = build_bass()

    pred = np.ascontiguousarray(pred, dtype=np.float32).reshape(B_FULL, CJ)
    target = np.ascontiguousarray(target, dtype=np.float32).reshape(B_FULL, CJ)

    in_maps = []
    for c in range(N_CORES):
        sl = slice(c * B_LOC, (c + 1) * B_LOC)
        in_maps.append({"pred": pred[sl], "target": target[sl]})

    res = run_bass_kernel_spmd(_NC, in_maps, core_ids=list(range(N_CORES)))
    mp = pa = ac = 0.0
    for r in res.results:
        o = r["out"].astype(np.float64)
        mp += o[:, 0:NACC].sum()
        pa += o[:, NACC:2 * NACC].sum()
        ac += o[:, 2 * NACC:3 * NACC].sum()
    inv = 1.0 / SCALE
    return np.array([mp / (B_FULL * 14) * inv,
                     pa / (B_FULL * 14) * inv,
                     ac / (B_FULL * 12) * inv], dtype=np.float32)


# revision 35
# speedup vs baseline: 1.0719x; 1.0338x over previous
"""PoseMetrics (mpjpe / pa_mpjpe / accel_error) Trainium2 Bass kernel.

Full inputs: pred/target [524288, 3, 14] fp32. Output: [3] fp32.

Strategy (pure data parallel, 8 cores x 65536 samples):
  - Layout: 128 partitions x 512 samples/partition, samples innermost so the
    bulk fp16 elementwise work hits the DVE 2x mode. Inputs are converted
    once to persistent fp16 SBUF tiles (with a global 1/sqrt(8) prescale) and
    never re-streamed.
  - The tensor engine (PE) acts as a free accumulator: identity-weight
    matmuls into PSUM replace the j-sum trees (cross-covariance G, joint sums
    SP/ST) and the 3-way coordinate sums for the per-joint norms.
  - Kabsch/SVD is replaced by a closed form: K = H^T H, largest eigenvalue
    via cubic Newton (Cardano-bound start, 2 iters), remaining eigenvalues by
    quadratic deflation, lambda = s1+s2+sign(det H)*s3, then Markley's FOAM
    formula for R. Slab math fp16, eigen chain fp32.
  - Each core returns [128, 48] partial sums; host reduces in float64.
"""

import numpy as np

import concourse.bass as bass
import concourse.bacc as bacc
import concourse.mybir as mybir
import concourse.tile as tile
from concourse.bass_utils import run_bass_kernel_spmd
from concourse.masks import make_identity

F32 = mybir.dt.float32
F16 = mybir.dt.float16
AX = mybir.AluOpType
AF = mybir.ActivationFunctionType

N_CORES = 8
B_FULL = 524288
B_LOC = B_FULL // N_CORES          # 65536
P = 128                            # partitions
S = B_LOC // P                     # 512 samples per partition
NB = 64                            # samples per chunk (per partition)
NCHUNK = S // NB                   # 8
CJ = 42                            # 3*14
SF = 256                           # FOAM half size
SCALE = float(1.0 / np.sqrt(8.0))  # global input prescale (folded out on host)
SQ14I = float(1.0 / np.sqrt(14.0))
NACC = NCHUNK                      # accum slots per metric (1 per chunk)


def _pass1_chunk(nc, loadp, workp, pv, tv, p16, t16, Gp2, Gp1, SPp, STp,
                 n2M, n2A, accM, accA, G16, SPh, STh, I16, ci):
    cs = slice(ci * NB, (ci + 1) * NB)
    x32p = loadp.tile([P, NB, CJ], F32, tag="p32", name="x32p")
    x32t = loadp.tile([P, NB, CJ], F32, tag="t32", name="x32t")
    # fp32 -> fp16 J-major convert with the global prescale folded in.
    # On Pool: ACT is the pass-1 critical engine, Pool is idle here.
    # Chunk 0 is split into quarters so compute starts ~6us earlier.
    nsub = 4 if ci == 0 else 1
    sw = NB // nsub
    for si in range(nsub):
        ls = slice(si * sw, si * sw + sw)
        gs = slice(ci * NB + si * sw, ci * NB + si * sw + sw)
        nc.sync.dma_start(x32p[:, ls, :], pv[:, gs, :])
        nc.sync.dma_start(x32t[:, ls, :], tv[:, gs, :])
        nc.gpsimd.tensor_scalar_mul(
            p16[:, :, :, gs],
            x32p[:, ls, :].rearrange("p s (c j) -> p c j s", c=3, j=14), SCALE)
        nc.gpsimd.tensor_scalar_mul(
            t16[:, :, :, gs],
            x32t[:, ls, :].rearrange("p s (c j) -> p c j s", c=3, j=14), SCALE)

    pcs = p16[:, :, :, cs]
    tcs = t16[:, :, :, cs]

    # ---- mpjpe: d, d^2, PE c-sum, sqrt-accum --------------------------------
    d = workp.tile([P, 3, 14, NB], F16, tag="d", name="d")
    nc.vector.tensor_tensor(d[:], pcs, tcs, op=AX.subtract)
    d2 = workp.tile([P, 3, 14, NB], F16, tag="d2", name="d2")
    nc.scalar.square(d2[:], d[:])
    for sub in range(2):
        ss = slice(sub * 32, sub * 32 + 32)
        ov = n2M[:][:, ss, 0:14].transpose([0, 2, 1])
        for c in range(3):
            nc.tensor.matmul(ov, I16[:], d2[:, c, :, ss],
                             start=(c == 0), stop=(c == 2))
    scrM = workp.tile([P, 64, 14], F16, tag="scrM", name="scrM")
    nc.scalar.activation(scrM[:], n2M[:][:, :, 0:14], AF.Sqrt,
                         accum_out=accM[:, ci:ci + 1])

    # ---- accel: second difference over j, squares, PE c-sum ----------------
    ta = workp.tile([P, 3, 12, NB], F16, tag="ta", name="ta")
    nc.vector.tensor_scalar_mul(ta[:], pcs[:, :, 1:13, :], -2.0)
    nc.vector.tensor_tensor(ta[:], ta[:], pcs[:, :, 0:12, :], op=AX.add)
    nc.vector.tensor_tensor(ta[:], ta[:], pcs[:, :, 2:14, :], op=AX.add)
    a2 = workp.tile([P, 3, 12, NB], F16, tag="a2", name="a2")
    nc.scalar.square(a2[:], ta[:])
    for sub in range(2):
        ss = slice(sub * 32, sub * 32 + 32)
        ov = n2A[:][:, ss, 0:12].transpose([0, 2, 1])
        for c in range(3):
            nc.tensor.matmul(ov, I16[:], a2[:, c, :, ss],
                             start=(c == 0), stop=(c == 2))
    scrA = workp.tile([P, 64, 12], F16, tag="scrA", name="scrA")
    nc.scalar.activation(scrA[:], n2A[:][:, :, 0:12], AF.Sqrt,
                         accum_out=accA[:, ci:ci + 1])

    # ---- G / SP / ST via PE -------------------------------------------------
    # prod[k, i, j, s] = p_i t_k; one TT per k keeps APs within 3 free dims.
    CS = [P, 3, 14, NB]
    prod = workp.tile([P, 3, 3, 14, NB], F16, tag="prod", name="prod")
    for k in range(3):
        nc.vector.tensor_tensor(
            prod[:, k], pcs,
            tcs[:, k].unsqueeze(1).broadcast_to(CS), op=AX.mult)
    # G16[k, i] = sum_j prod[k, i, j]; split k to fit PSUM banks
    for (gp, ksl, nk) in ((Gp2, slice(0, 2), 2), (Gp1, slice(2, 3), 1)):
        for j in range(14):
            nc.tensor.matmul(gp[:], I16[:], prod[:, ksl, :, j, :],
                             start=(j == 0), stop=(j == 13))
    for j in range(14):
        nc.tensor.matmul(SPp[:], I16[:], p16[:, :, j, cs],
                         start=(j == 0), stop=(j == 13))
    for j in range(14):
        nc.tensor.matmul(STp[:], I16[:], t16[:, :, j, cs],
                         start=(j == 0), stop=(j == 13))

    # drains: G + SP/ST on ACT (GPSIMD cannot read PSUM)
    nc.scalar.copy(G16[:, 0:2, :, cs], Gp2[:])
    nc.scalar.copy(G16[:, 2:3, :, cs], Gp1[:])
    nc.scalar.activation(SPh[:, :, cs], SPp[:], AF.Copy, scale=SQ14I)
    nc.scalar.activation(STh[:, :, cs], STp[:], AF.Copy, scale=SQ14I)


def _foam_inv(nc, sp_, chp, G16, SPh, STh, R16, V16, hf):
    """FOAM part A1: H, K, det, invariants.

    H is in s^2 = 1/8 scale (inherited from the input prescale); the FOAM
    formula is scale-invariant so no rescaling is needed anywhere.
    SPh/STh are joint sums scaled by 1/sqrt(14).
    """
    fs = slice(hf * SF, hf * SF + SF)
    S3 = [P, 3, 3, SF]
    # G16 is stored (k, i); present it as (i, k) via a stride view
    Gv = G16[:, :, :, fs].transpose([0, 2, 1, 3])
    SPv = SPh[:, :, fs]
    STv = STh[:, :, fs]

    def slab(name):
        # rotating scratch slab; at most `bufs` of these live at once
        return sp_.tile(S3, F16, tag="ktmp", name=name)

    def ch(name, dt=F32):
        return chp.tile([P, SF], dt, tag="ch32" if dt == F32 else "ch16",
                        name=name)

    def named(tag, dt=F32):
        return chp.tile([P, SF], dt, tag=tag, name=tag, bufs=1)

    # H = G - SP ST^T / 14  (SPh*STh = SP*ST/14 already)
    outer = slab("outer")
    nc.vector.tensor_tensor(
        outer[:], SPv.unsqueeze(2).broadcast_to(S3),
        STv.unsqueeze(1).broadcast_to(S3), op=AX.mult)
    H16 = sp_.tile(S3, F16, tag="H16", bufs=1, name="H16")
    nc.vector.tensor_tensor(H16[:], Gv, outer[:], op=AX.subtract)

    # detH on Pool (fp32 out), from fp16 H
    detH = named("detH")
    c1 = ch("det_c1"); c2 = ch("det_c2"); acc = ch("det_acc")
    nc.gpsimd.tensor_tensor(c1[:], H16[:, 1, 1], H16[:, 2, 2], op=AX.mult)
    nc.gpsimd.tensor_tensor(c2[:], H16[:, 1, 2], H16[:, 2, 1], op=AX.mult)
    nc.gpsimd.tensor_tensor(c1[:], c1[:], c2[:], op=AX.subtract)
    nc.gpsimd.tensor_tensor(acc[:], H16[:, 0, 0], c1[:], op=AX.mult)
    nc.gpsimd.tensor_tensor(c1[:], H16[:, 1, 0], H16[:, 2, 2], op=AX.mult)
    nc.gpsimd.tensor_tensor(c2[:], H16[:, 1, 2], H16[:, 2, 0], op=AX.mult)
    nc.gpsimd.tensor_tensor(c1[:], c1[:], c2[:], op=AX.subtract)
    nc.gpsimd.tensor_tensor(c1[:], H16[:, 0, 1], c1[:], op=AX.mult)
    nc.gpsimd.tensor_tensor(acc[:], acc[:], c1[:], op=AX.subtract)
    nc.gpsimd.tensor_tensor(c1[:], H16[:, 1, 0], H16[:, 2, 1], op=AX.mult)
    nc.gpsimd.tensor_tensor(c2[:], H16[:, 1, 1], H16[:, 2, 0], op=AX.mult)
    nc.gpsimd.tensor_tensor(c1[:], c1[:], c2[:], op=AX.subtract)
    nc.gpsimd.tensor_tensor(c1[:], H16[:, 0, 2], c1[:], op=AX.mult)
    nc.gpsimd.tensor_tensor(detH[:], acc[:], c1[:], op=AX.add)

    # K = H^T H (fp16 slabs, accumulate into K16)
    K16 = sp_.tile(S3, F16, tag="K16", bufs=1, name="K16")
    nc.vector.tensor_tensor(K16[:], H16[:, 0].unsqueeze(2).broadcast_to(S3),
                            H16[:, 0].unsqueeze(1).broadcast_to(S3), op=AX.mult)
    for c in (1, 2):
        tc_ = slab(f"t{c}")
        nc.vector.tensor_tensor(tc_[:], H16[:, c].unsqueeze(2).broadcast_to(S3),
                                H16[:, c].unsqueeze(1).broadcast_to(S3),
                                op=AX.mult)
        nc.vector.tensor_tensor(K16[:], K16[:], tc_[:], op=AX.add)

    # invariants: m2 = tr K (fp32), I3 = detH^2, I2 via Pool
    m2 = named("m2")
    nc.vector.tensor_tensor(m2[:], K16[:, 0, 0], K16[:, 1, 1], op=AX.add)
    nc.vector.tensor_tensor(m2[:], m2[:], K16[:, 2, 2], op=AX.add)
    I3 = named("I3")
    nc.vector.tensor_tensor(I3[:], detH[:], detH[:], op=AX.mult)

    o01 = ch("o01"); o02 = ch("o02"); o12 = ch("o12")
    nc.scalar.square(o01[:], K16[:, 0, 1])
    nc.scalar.square(o02[:], K16[:, 0, 2])
    nc.scalar.square(o12[:], K16[:, 1, 2])
    I2 = named("I2"); mm = ch("mm")
    nc.gpsimd.tensor_tensor(I2[:], K16[:, 0, 0], K16[:, 1, 1], op=AX.mult)
    nc.gpsimd.tensor_tensor(I2[:], I2[:], o01[:], op=AX.subtract)
    nc.gpsimd.tensor_tensor(mm[:], K16[:, 0, 0], K16[:, 2, 2], op=AX.mult)
    nc.gpsimd.tensor_tensor(mm[:], mm[:], o02[:], op=AX.subtract)
    nc.gpsimd.tensor_tensor(I2[:], I2[:], mm[:], op=AX.add)
    nc.gpsimd.tensor_tensor(mm[:], K16[:, 1, 1], K16[:, 2, 2], op=AX.mult)
    nc.gpsimd.tensor_tensor(mm[:], mm[:], o12[:], op=AX.subtract)
    nc.gpsimd.tensor_tensor(I2[:], I2[:], mm[:], op=AX.add)

    return {"H16": H16, "K16": K16, "detH": detH, "m2": m2, "I2": I2,
            "I3": I3, "o01": o01, "o02": o02, "o12": o12}


def _foam_chain(nc, sp_, chp, st, hf):
    """FOAM part A2: eigen chain (Cardano start, Newton, deflation), adjH."""
    fs = slice(hf * SF, hf * SF + SF)
    S3 = [P, 3, 3, SF]
    H16 = st["H16"]; K16 = st["K16"]; detH = st["detH"]
    m2 = st["m2"]; I2 = st["I2"]; I3 = st["I3"]
    o01 = st["o01"]; o02 = st["o02"]; o12 = st["o12"]

    def ch(name, dt=F32):
        return chp.tile([P, SF], dt, tag="ch32" if dt == F32 else "ch16",
                        name=name)

    def named(tag, dt=F32):
        return chp.tile([P, SF], dt, tag=tag, name=tag, bufs=1)

    # Cardano upper bound start: x0 = m2/3 + 2*sqrt((dsum + 2*osum)/6)
    q = named("q")
    nc.vector.tensor_scalar_mul(q[:], m2[:], 1.0 / 3.0)
    osum = ch("osum")
    nc.vector.tensor_tensor(osum[:], o01[:], o02[:], op=AX.add)
    nc.vector.tensor_tensor(osum[:], osum[:], o12[:], op=AX.add)
    dsum = ch("dsum"); kd = ch("kd"); kd2 = ch("kd2")
    nc.vector.tensor_tensor(kd[:], K16[:, 0, 0], q[:], op=AX.subtract)
    nc.vector.tensor_tensor(dsum[:], kd[:], kd[:], op=AX.mult)
    nc.vector.tensor_tensor(kd[:], K16[:, 1, 1], q[:], op=AX.subtract)
    nc.vector.tensor_tensor(kd2[:], kd[:], kd[:], op=AX.mult)
    nc.vector.tensor_tensor(dsum[:], dsum[:], kd2[:], op=AX.add)
    nc.vector.tensor_tensor(kd[:], K16[:, 2, 2], q[:], op=AX.subtract)
    nc.vector.tensor_tensor(kd2[:], kd[:], kd[:], op=AX.mult)
    nc.vector.tensor_tensor(dsum[:], dsum[:], kd2[:], op=AX.add)
    p2 = ch("p2")
    nc.vector.scalar_tensor_tensor(p2[:], osum[:], 2.0, dsum[:],
                                   op0=AX.mult, op1=AX.add)
    pC = ch("pC")
    nc.scalar.activation(pC[:], p2[:], AF.Sqrt, scale=1.0 / 6.0)
    X = named("X")
    nc.vector.scalar_tensor_tensor(X[:], pC[:], 2.0, q[:],
                                   op0=AX.mult, op1=AX.add)

    # Newton on f(x) = ((x - m2) x + I2) x - I3, 2 iters from above
    m2_2 = named("m2_2")
    nc.vector.tensor_scalar_mul(m2_2[:], m2[:], 2.0)
    na = ch("na"); nb = ch("nb")
    for _ in range(1):
        nc.vector.tensor_tensor(na[:], X[:], m2[:], op=AX.subtract)
        nc.vector.tensor_tensor(na[:], na[:], X[:], op=AX.mult)
        nc.vector.tensor_tensor(na[:], na[:], I2[:], op=AX.add)
        nc.vector.tensor_tensor(na[:], na[:], X[:], op=AX.mult)
        nc.vector.tensor_tensor(na[:], na[:], I3[:], op=AX.subtract)   # f
        nc.vector.tensor_scalar_mul(nb[:], X[:], 3.0)
        nc.vector.tensor_tensor(nb[:], nb[:], m2_2[:], op=AX.subtract)
        nc.vector.tensor_tensor(nb[:], nb[:], X[:], op=AX.mult)
        nc.vector.tensor_tensor(nb[:], nb[:], I2[:], op=AX.add)        # f'
        nc.vector.reciprocal(nb[:], nb[:])
        nc.vector.tensor_tensor(na[:], na[:], nb[:], op=AX.mult)
        nc.vector.tensor_tensor(X[:], X[:], na[:], op=AX.subtract)

    # deflate: mu2/mu3 from x^2 - (m2-mu1)x + I3/mu1
    mus = chp.tile([P, 3, SF], F32, tag="mus", name="mus", bufs=1)
    mu1 = mus[:, 0]; mu2 = mus[:, 1]; mu3 = mus[:, 2]
    nc.vector.tensor_scalar_max(mu1, X[:], 1e-25)
    b = ch("b"); cc = ch("cc"); rmu = ch("rmu")
    nc.vector.tensor_tensor(b[:], m2[:], mu1, op=AX.subtract)
    nc.vector.reciprocal(rmu[:], mu1)
    nc.vector.tensor_tensor(cc[:], I3[:], rmu[:], op=AX.mult)
    b2 = ch("b2")
    nc.vector.tensor_tensor(b2[:], b[:], b[:], op=AX.mult)
    disc2 = ch("disc2")
    nc.vector.scalar_tensor_tensor(disc2[:], cc[:], -4.0, b2[:],
                                   op0=AX.mult, op1=AX.add)
    nc.vector.tensor_scalar_max(disc2[:], disc2[:], 0.0)
    disc = ch("disc")
    nc.scalar.sqrt(disc[:], disc2[:])
    bh = ch("bh")
    nc.vector.tensor_scalar_mul(bh[:], b[:], 0.5)
    nc.vector.scalar_tensor_tensor(mu2, disc[:], 0.5, bh[:],
                                   op0=AX.mult, op1=AX.add)
    nc.vector.tensor_scalar_max(mu2, mu2, 0.0)
    nc.vector.tensor_tensor(mu3, b[:], mu2, op=AX.subtract)
    nc.vector.tensor_scalar_max(mu3, mu3, 0.0)

    rt = chp.tile([P, 3, SF], F32, tag="rt", name="rt", bufs=1)
    nc.scalar.sqrt(rt[:], mus[:])
    sgn = ch("sgn")
    nc.scalar.sign(sgn[:], detH[:])
    lam = named("lam")
    nc.vector.tensor_tensor(lam[:], rt[:, 0], rt[:, 1], op=AX.add)
    s3s = ch("s3s")
    nc.vector.tensor_tensor(s3s[:], sgn[:], rt[:, 2], op=AX.mult)
    nc.vector.tensor_tensor(lam[:], lam[:], s3s[:], op=AX.add)

    # alpha2 = lam^2 + m2 ; zeta2 = (lam^2 - m2) lam - 2 detH (floored)
    lam2 = ch("lam2"); alpha2 = named("alpha2")
    nc.vector.tensor_tensor(lam2[:], lam[:], lam[:], op=AX.mult)
    nc.vector.tensor_tensor(alpha2[:], lam2[:], m2[:], op=AX.add)
    zt = ch("zt")
    nc.vector.tensor_tensor(zt[:], lam2[:], m2[:], op=AX.subtract)
    nc.vector.tensor_tensor(zt[:], zt[:], lam[:], op=AX.mult)
    zeta2 = ch("zeta2")
    nc.vector.scalar_tensor_tensor(zeta2[:], detH[:], -2.0, zt[:],
                                   op0=AX.mult, op1=AX.add)
    m2s = ch("m2s")
    nc.scalar.sqrt(m2s[:], m2[:])
    zfl = ch("zfl")
    nc.vector.scalar_tensor_tensor(zfl[:], m2s[:], 1e-4, m2[:],
                                   op0=AX.mult, op1=AX.mult)
    nc.vector.tensor_tensor(zeta2[:], zeta2[:], zfl[:], op=AX.max)
    rz = ch("rz")
    nc.vector.reciprocal(rz[:], zeta2[:])

    # fp16 stage for the slab assembly
    a16 = named("a16", F16)
    nc.vector.tensor_copy(a16[:], alpha2[:])
    l16 = named("l16", F16)
    nc.vector.tensor_scalar_mul(l16[:], lam[:], 2.0)
    rz16 = named("rz16", F16)
    nc.vector.tensor_copy(rz16[:], rz[:])

    # adjugate of H: fp16 channel ops on DVE (cheap in 2x mode)
    adjH = sp_.tile(S3, F16, tag="adjH", bufs=1, name="adjH")
    idx = [
        (0, 0, (1, 1), (2, 2), (1, 2), (2, 1)),
        (0, 1, (0, 2), (2, 1), (0, 1), (2, 2)),
        (0, 2, (0, 1), (1, 2), (0, 2), (1, 1)),
        (1, 0, (1, 2), (2, 0), (1, 0), (2, 2)),
        (1, 1, (0, 0), (2, 2), (0, 2), (2, 0)),
        (1, 2, (0, 2), (1, 0), (0, 0), (1, 2)),
        (2, 0, (1, 0), (2, 1), (1, 1), (2, 0)),
        (2, 1, (0, 1), (2, 0), (0, 0), (2, 1)),
        (2, 2, (0, 0), (1, 1), (0, 1), (1, 0)),
    ]
    aw1 = ch("aw1", F16); aw2 = ch("aw2", F16)
    for (i, j, (a1, a2), (b1, b2), (c1_, c2_), (d1, d2)) in idx:
        nc.vector.tensor_tensor(aw1[:], H16[:, a1, a2], H16[:, b1, b2], op=AX.mult)
        nc.vector.tensor_tensor(aw2[:], H16[:, c1_, c2_], H16[:, d1, d2], op=AX.mult)
        nc.vector.tensor_tensor(adjH[:, i, j], aw1[:], aw2[:], op=AX.subtract)

    return {"H16": H16, "K16": K16, "adjH": adjH,
            "a16": a16, "l16": l16, "rz16": rz16}


def _foam_b(nc, sp_, chp, st, G16, SPh, STh, R16, V16, t16, hf):
    return _foam_half_b(nc, sp_, chp, st, G16, SPh, STh, R16, V16, t16, hf)


def _foam_half_b(nc, sp_, chp, st, G16, SPh, STh, R16, V16, t16, hf):
    """FOAM part B: slab assembly, R, V, and the V fold into t16."""
    fs = slice(hf * SF, hf * SF + SF)
    S3 = [P, 3, 3, SF]
    SPv = SPh[:, :, fs]
    STv = STh[:, :, fs]
    H16 = st["H16"]; K16 = st["K16"]; adjH = st["adjH"]
    a16 = st["a16"]; l16 = st["l16"]; rz16 = st["rz16"]

    def slab(name):
        return sp_.tile(S3, F16, tag="ktmp", name=name)

    # num = (alpha2 I - 2K) H^T + 2 lam adjH ;  R = num / zeta2, clamped
    W = sp_.tile(S3, F16, tag="Mt", bufs=1, name="W")
    nc.vector.tensor_scalar_mul(W[:], K16[:], -2.0)
    # diagonal view: stride 4*SF within the contiguous [3,3,SF] block
    nc.vector.tensor_tensor(
        W[:].rearrange("p a b s -> p (a b) s")[:, 0:9:4, :],
        W[:].rearrange("p a b s -> p (a b) s")[:, 0:9:4, :],
        a16[:].unsqueeze(1).broadcast_to([P, 3, SF]), op=AX.add)
    Ht = H16[:].transpose([0, 2, 1, 3])
    num = slab("num")
    nc.vector.tensor_tensor(num[:], W[:, :, 0].unsqueeze(2).broadcast_to(S3),
                            H16[:, :, 0].unsqueeze(1).broadcast_to(S3), op=AX.mult)
    for c in (1, 2):
        uc = slab(f"u{c}")
        nc.vector.tensor_tensor(uc[:], W[:, :, c].unsqueeze(2).broadcast_to(S3),
                                H16[:, :, c].unsqueeze(1).broadcast_to(S3),
                                op=AX.mult)
        nc.vector.tensor_tensor(num[:], num[:], uc[:], op=AX.add)
    vB = slab("vB")
    nc.vector.tensor_tensor(
        vB[:], l16[:].unsqueeze(1).unsqueeze(2).broadcast_to(S3), adjH[:],
        op=AX.mult)
    nc.vector.tensor_tensor(num[:], num[:], vB[:], op=AX.add)
    R16v = R16[:, :, :, fs]
    nc.vector.tensor_tensor(
        R16v, num[:], rz16[:].unsqueeze(1).unsqueeze(2).broadcast_to(S3),
        op=AX.mult)
    nc.vector.tensor_scalar(R16v, R16v, 4.0, -4.0, op0=AX.min, op1=AX.max)

    # V = (STh - R SPh) / sqrt(14)  (== t_mean - R p_mean)
    pv_ = slab("pv_")
    nc.vector.tensor_tensor(pv_[:], R16v, SPv.unsqueeze(1).broadcast_to(S3),
                            op=AX.mult)
    RS = chp.tile([P, 3, SF], F16, tag="RS", name="RS", bufs=1)
    nc.vector.tensor_tensor(RS[:], pv_[:, :, 0], pv_[:, :, 1], op=AX.add)
    nc.vector.tensor_tensor(RS[:], RS[:], pv_[:, :, 2], op=AX.add)
    Vt = chp.tile([P, 3, SF], F16, tag="Vt", name="Vt", bufs=1)
    nc.vector.tensor_tensor(Vt[:], STv, RS[:], op=AX.subtract)
    nc.vector.tensor_scalar_mul(V16[:, :, fs], Vt[:], SQ14I)

    # fold V into t16 in place: pass3's residual becomes qv - t16
    TSH = [P, 3, 14, SF]
    nc.vector.tensor_tensor(
        t16[:, :, :, fs], t16[:, :, :, fs],
        V16[:, :, fs].unsqueeze(2).broadcast_to(TSH), op=AX.subtract)


def _pass3_chunk(nc, workp, p16, t16, R16, V16, n2P, dvps, accP, I16, nI16,
                 ci, use_pe):
    cs = slice(ci * NB, (ci + 1) * NB)
    CS = [P, 3, 14, NB]
    # prq[k][i, j, s] = R_ik p_kj
    prqs = []
    for k in range(3):
        prq = workp.tile(CS, F16, tag=f"prq{k}", name=f"prq{k}")
        nc.vector.tensor_tensor(
            prq[:], R16[:, :, k, cs].unsqueeze(2).broadcast_to(CS),
            p16[:, k, :, cs].unsqueeze(1).broadcast_to(CS), op=AX.mult)
        prqs.append(prq)
    dv2 = workp.tile(CS, F16, tag="dv2", name="dv2")
    tcs = t16[:, :, :, cs]
    if use_pe:
        # PE sums over k and subtracts t16 (V already folded into t16)
        subs = [(i * 12, min(12, NB - i * 12)) for i in range((NB + 11) // 12)]
        for si, (s0, sw) in enumerate(subs):
            ss = slice(s0, s0 + sw)
            dvp = dvps[si % len(dvps)]
            for k in range(3):
                nc.tensor.matmul(dvp[:, :, :, 0:sw], I16[:],
                                 prqs[k][:, :, :, ss],
                                 start=(k == 0), stop=False)
            nc.tensor.matmul(dvp[:, :, :, 0:sw], nI16[:], tcs[:, :, :, ss],
                             start=False, stop=True)
            nc.scalar.square(dv2[:, :, :, ss], dvp[:, :, :, 0:sw])
    else:
        # DVE sums (tail chunks: PE is the critical engine there)
        nc.vector.tensor_tensor(prqs[0][:], prqs[0][:], prqs[1][:], op=AX.add)
        nc.vector.tensor_tensor(prqs[0][:], prqs[0][:], prqs[2][:], op=AX.add)
        nc.vector.tensor_tensor(prqs[0][:], prqs[0][:], tcs, op=AX.subtract)
        nc.scalar.square(dv2[:], prqs[0][:])
    for sub in range(2):
        ss = slice(sub * 32, sub * 32 + 32)
        ov = n2P[:][:, ss, 0:14].transpose([0, 2, 1])
        for c in range(3):
            nc.tensor.matmul(ov, I16[:], dv2[:, c, :, ss],
                             start=(c == 0), stop=(c == 2))
    scrP = workp.tile([P, 64, 14], F16, tag="scrP", name="scrP")
    nc.scalar.activation(scrP[:], n2P[:][:, :, 0:14], AF.Sqrt,
                         accum_out=accP[:, ci:ci + 1])


def build_bass():
    nc = bacc.Bacc("TRN2")
    pred = nc.dram_tensor("pred", [B_LOC, CJ], F32, kind="ExternalInput")
    targ = nc.dram_tensor("target", [B_LOC, CJ], F32, kind="ExternalInput")
    out = nc.dram_tensor("out", [P, 3 * NACC], F32, kind="ExternalOutput")

    pv = pred[:].rearrange("(p n) d -> p n d", p=P)   # [128, 512, 42]
    tv = targ[:].rearrange("(p n) d -> p n d", p=P)

    with tile.TileContext(nc) as tc:
        with tc.tile_pool(name="persist", bufs=1) as pp:
            p16 = pp.tile([P, 3, 14, S], F16, tag="p16")
            t16 = pp.tile([P, 3, 14, S], F16, tag="t16")
            G16 = pp.tile([P, 3, 3, S], F16, tag="G16")
            SPh = pp.tile([P, 3, S], F16, tag="SPh")
            STh = pp.tile([P, 3, S], F16, tag="STh")
            R16 = pp.tile([P, 3, 3, S], F16, tag="R16")
            V16 = pp.tile([P, 3, S], F16, tag="V16")
            accM = pp.tile([P, NACC], F32, tag="accM")
            accA = pp.tile([P, NACC], F32, tag="accA")
            accP = pp.tile([P, NACC], F32, tag="accP")
            I16 = pp.tile([P, P], F16, tag="I16")
            make_identity(nc, I16[:])
            nI16 = pp.tile([P, P], F16, tag="nI16")
            nc.vector.tensor_scalar_mul(nI16[:], I16[:], -1.0)

            # ---------------- pass 1 ----------------------------------------
            with tc.tile_pool(name="load1", bufs=2) as loadp, \
                 tc.tile_pool(name="work1", bufs=1) as workp, \
                 tc.tile_pool(name="ps1", bufs=1, space="PSUM") as psp:
                Gp2 = psp.tile([P, 2, 3, NB], F32, tag="Gp2")
                Gp1 = psp.tile([P, 1, 3, NB], F32, tag="Gp1")
                SPp = psp.tile([P, 3, NB], F32, tag="SPp")
                STp = psp.tile([P, 3, NB], F32, tag="STp")
                n2M = psp.tile([P, 64, 16], F32, tag="n2M", name="n2M")
                n2A = psp.tile([P, 64, 16], F32, tag="n2A", name="n2A")
                for ci in range(NCHUNK):
                    _pass1_chunk(nc, loadp, workp, pv, tv, p16, t16,
                                 Gp2, Gp1, SPp, STp, n2M, n2A,
                                 accM, accA, G16, SPh, STh, I16, ci)

            # ---------------- FOAM + pass 3, interleaved --------------------
            with tc.tile_pool(name="slab_a", bufs=2) as sp_a, \
                 tc.tile_pool(name="ch_a", bufs=14) as chp_a, \
                 tc.tile_pool(name="work3", bufs=1) as workp3, \
                 tc.tile_pool(name="ps3", bufs=1, space="PSUM") as psp3:
                n2P = psp3.tile([P, 64, 16], F32, tag="n2P", name="n2P")
                dvps = [psp3.tile([P, 3, 14, 12], F32, tag=f"dvp{s}",
                                  name=f"dvp{s}") for s in range(2)]
                def P3(ci, use_pe=True):
                    _pass3_chunk(nc, workp3, p16, t16, R16, V16, n2P, dvps,
                                 accP, I16, nI16, ci, use_pe)
                st0 = _foam_inv(nc, sp_a, chp_a, G16, SPh, STh, R16, V16, 0)
                st0 = dict(st0, **_foam_chain(nc, sp_a, chp_a, st0, 0))
                _foam_b(nc, sp_a, chp_a, st0, G16, SPh, STh, R16, V16, t16, 0)
                st1 = _foam_inv(nc, sp_a, chp_a, G16, SPh, STh, R16, V16, 1)
                st1 = dict(st1, **_foam_chain(nc, sp_a, chp_a, st1, 1))
                for ci in range(4):
                    P3(ci)
                _foam_b(nc, sp_a, chp_a, st1, G16, SPh, STh, R16, V16, t16, 1)
                for ci in range(4, NCHUNK):
                    P3(ci)

            stage = pp.tile([P, 3 * NACC], F32, tag="stage", name="stage")
            nc.gpsimd.tensor_copy(stage[:, 0:NACC], accM[:])
            nc.gpsimd.tensor_copy(stage[:, NACC:2 * NACC], accP[:])
            nc.gpsimd.tensor_copy(stage[:, 2 * NACC:3 * NACC], accA[:])
            nc.sync.dma_start(out[:], stage[:])

    nc.finalize()
    return nc


_NC = None


def kernel(pred: np.ndarray, target: np.ndarray) -> np.ndarray:
    global _NC
    if _NC is None:
        _NC = build_bass()

    pred = np.ascontiguousarray(pred, dtype=np.float32).reshape(B_FULL, CJ)
    target = np.ascontiguousarray(target, dtype=np.float32).reshape(B_FULL, CJ)

    in_maps = []
    for c in range(N_CORES):
        sl = slice(c * B_LOC, (c + 1) * B_LOC)
        in_maps.append({"pred": pred[sl], "target": target[sl]})

    res = run_bass_kernel_spmd(_NC, in_maps, core_ids=list(range(N_CORES)))
    mp = pa = ac = 0.0
    for r in res.results:
        o = r["out"].astype(np.float64)
        mp += o[:, 0:NACC].sum()
        pa += o[:, NACC:2 * NACC].sum()
        ac += o[:, 2 * NACC:3 * NACC].sum()
    inv = 1.0 / SCALE
    return np.array([mp / (B_FULL * 14) * inv,
                     pa / (B_FULL * 14) * inv,
                     ac / (B_FULL * 12) * inv], dtype=np.float32)
